# revision 22
# baseline (speedup 1.0000x reference)
"""Trainium2 Bass kernel for nn_CapRNNModelHelper (bi-GRU + capsule routing).

Sharding: data-parallel over batch across 8 cores (16 batch rows per core).
Everything else (embedding table, GRU weights, capsule weights) replicated.

Per-core pipeline:
  1. indirect-DMA gather of embedding rows (token order s-major)
  2. PE-transpose -> e.T  [300, ntok]
  3. x_proj matmuls -> xp [128, S, 96]  (psum col blocks [rf zf rb zb | nf nb],
     z blocks negated so sigmoid gives w = 1-z directly)
  4. 256-step fused bidirectional GRU scan (H on partitions, dirs+batch on free)
  5. capsule matmul -> u_hat [sb, 160]
  6. 5-iter dynamic routing (selector matmuls for sequence reductions)
  7. final linear -> out [16, 2]
"""

import numpy as np
from contextlib import ExitStack

import concourse.bass as bass
import concourse.tile as tile
from concourse import mybir
from concourse.bass import IndirectOffsetOnAxis
from concourse.bass_utils import run_bass_kernel_spmd

F32 = mybir.dt.float32
I32 = mybir.dt.int32
AF = mybir.ActivationFunctionType
OP = mybir.AluOpType
AX = mybir.AxisListType

VOCAB, D_W, H, S, B = 50000, 300, 128, 256, 128
NUM_CAP, DIM_CAP, ROUTINGS, EPS = 10, 16, 5, 1e-7
NCORES = 8
BL = B // NCORES          # 16 batch rows per core
NTOK = S * BL             # 4096 tokens per core
NGRP = NTOK // 128        # 32 gather groups of 128 tokens
NCH = NTOK // 512         # 8 x_proj chunks of 512 tokens
KCH = [(0, 128), (128, 128), (256, 44)]   # D_W split
G3 = 3 * H                # 384

# xp free layout per step: 6 blocks of BL: [r_f z_f r_b z_b n_f n_b]
XPW = 6 * BL              # 96
RZW = 4 * BL              # 64
NW = 2 * BL               # 32
# block index for (dir d, gate g): r/z interleaved by dir, n at end
_BLK = {(0, 0): 0, (0, 1): 1, (1, 0): 2, (1, 1): 3, (0, 2): 4, (1, 2): 5}


def _sub(base, off, dims):
    """Manual AP: base is a [128, X] AP; append free dims after partition."""
    return bass.AP(tensor=base.tensor, offset=base.offset + off,
                   ap=[base.ap[0]] + dims)


def _v(t, dims, off=0):
    """Free-dim view of a tile AP."""
    return bass.AP(tensor=t.tensor, offset=t.offset + off,
                   ap=[t.ap[0]] + dims)


def _split_waits(nc, cap=1):
    """Hoist excess sync waits onto standalone event-semaphore ops.

    The walrus build on this stack accepts only `cap` sync-wait commands
    per ISA instruction; Tile can attach several. Event-semaphore ops on
    the same engine execute in queue order, so hoisting preserves
    semantics.
    """
    n = 0
    for fn in nc.m.functions:
        for bb in fn.blocks:
            out = []
            for ins in bb.instructions:
                si = ins.sync_info
                if si is not None and len(si.on_wait) > cap:
                    waits = list(si.on_wait)
                    keep = waits[len(waits) - cap:] if cap else []
                    for w in waits[:len(waits) - cap] if cap else waits:
                        n += 1
                        out.append(mybir.InstEventSemaphore(
                            name=f"wsplit-{n}", engine=ins.engine,
                            ins=[], outs=[],
                            sync_info=mybir.SyncInfo(on_wait=[w],
                                                     on_update=[])))
                    ins.sync_info = mybir.SyncInfo(
                        on_wait=keep, on_update=list(si.on_update))
                out.append(ins)
            bb.instructions = out
    return n


def _build(zero_bhn: bool, debug: bool = False):
    nc = bass.Bass()
    if debug:
        dbg_xp_d = nc.declare_dram_parameter("dbg_xp", [128, S * XPW], F32, True)
        dbg_hs_d = nc.declare_dram_parameter("dbg_hs", [128, 2 * (S + 1) * BL],
                                             F32, True)
        dbg_uh_d = nc.declare_dram_parameter("dbg_uh", [128, NGRP * 160], F32,
                                             True)
        dbg_bl_d = nc.declare_dram_parameter("dbg_bl", [128, NGRP * NUM_CAP],
                                             F32, True)
        dbg_o_d = nc.declare_dram_parameter("dbg_o", [BL, 160], F32, True)

    xidx_d = nc.declare_dram_parameter("xidx", [128, NGRP], I32, False)
    emb_d = nc.declare_dram_parameter("emb", [VOCAB, D_W], F32, False)
    wih_d = nc.declare_dram_parameter("wih", [2, D_W, G3], F32, False)
    whh_d = nc.declare_dram_parameter("whh", [2, H, G3], F32, False)
    biasx_d = nc.declare_dram_parameter("biasx", [128, 6], F32, False)
    bhn_d = nc.declare_dram_parameter("bhn", [128, 2], F32, False)
    wcap_d = nc.declare_dram_parameter("wcap", [2, H, 160], F32, False)
    wlin_d = nc.declare_dram_parameter("wlin", [160, 2], F32, False)
    blin_d = nc.declare_dram_parameter("blin", [2, 1], F32, False)
    selB_d = nc.declare_dram_parameter("selB", [128, BL], F32, False)
    selT_d = nc.declare_dram_parameter("selT", [BL, 128], F32, False)
    ident_d = nc.declare_dram_parameter("ident", [128, 128], F32, False)
    out_d = nc.declare_dram_parameter("out", [BL, 2], F32, True)

    with tile.TileContext(nc) as tc, ExitStack() as ctx:
        const = ctx.enter_context(tc.tile_pool(name="const", bufs=1))
        bigxp = ctx.enter_context(tc.tile_pool(name="bigxp", bufs=1))
        bighs = ctx.enter_context(tc.tile_pool(name="bighs", bufs=1))
        work = ctx.enter_context(tc.tile_pool(name="work", bufs=3))

        # ---- constants to SBUF ----
        xidx = const.tile([128, NGRP], I32)
        nc.sync.dma_start(out=xidx[:], in_=xidx_d[:, :])
        whh = const.tile([128, 2, G3], F32)
        for d in range(2):
            nc.sync.dma_start(out=whh[:, d, :], in_=whh_d[d, :, :])
        biasx = const.tile([128, 6], F32)
        nc.sync.dma_start(out=biasx[:], in_=biasx_d[:, :])
        bhn = const.tile([128, 2], F32)
        nc.sync.dma_start(out=bhn[:], in_=bhn_d[:, :])
        wcap = const.tile([128, 2, 160], F32)
        for k in range(2):
            nc.sync.dma_start(out=wcap[:, k, :], in_=wcap_d[k, :, :])
        wlin = const.tile([128, 2, 2], F32)        # chunk0 [:128], chunk1 [:32]
        nc.sync.dma_start(out=wlin[:, 0, :], in_=wlin_d[0:128, :])
        nc.sync.dma_start(out=wlin[:32, 1, :], in_=wlin_d[128:160, :])
        blin = const.tile([2, 1], F32)
        nc.sync.dma_start(out=blin[:], in_=blin_d[:, :])
        selB = const.tile([128, BL], F32)
        nc.sync.dma_start(out=selB[:], in_=selB_d[:, :])
        selT = const.tile([BL, 128], F32)
        nc.sync.dma_start(out=selT[:], in_=selT_d[:, :])
        ident = const.tile([128, 128], F32)
        nc.sync.dma_start(out=ident[:], in_=ident_d[:, :])
        epst = const.tile([128, 1], F32)
        nc.vector.memset(epst[:], EPS)

        xp = bigxp.tile([128, S * XPW], F32)        # 98.3 KB/part
        HB0 = (S + 1) * BL
        hs = bighs.tile([128, 2 * (S + 1) * BL], F32)   # 32.9 KB/part

        # ---- phases B+C: gather + transpose + x_proj, in two half passes ----
        HTOK = NTOK // 2
        with tc.tile_pool(name="bc", bufs=1) as bc, \
             tc.tile_pool(name="gat", bufs=4) as gat, \
             tc.tile_pool(name="ps_bc", bufs=1, space="PSUM") as ps_bc:
            wih = bc.tile([128, 2, 3, G3], F32)    # [kpart, dir, kchunk, gatecol]
            for d in range(2):
                for k, (k0, kn) in enumerate(KCH):
                    nc.sync.dma_start(out=wih[:kn, d, k, :],
                                      in_=wih_d[d, k0:k0 + kn, :])
            for half in range(2):
                eT = [bc.tile([128, HTOK], F32, name=f"eT{k}", tag=f"eT{k}")
                      for k in range(3)]
                for i in range(NGRP // 2):
                    ig = half * (NGRP // 2) + i
                    g = gat.tile([128, D_W], F32, name="g", tag="g")
                    nc.gpsimd.indirect_dma_start(
                        out=g[:], out_offset=None,
                        in_=emb_d[:, :],
                        in_offset=IndirectOffsetOnAxis(ap=xidx[:, ig:ig + 1],
                                                       axis=0))
                    for k, (k0, kn) in enumerate(KCH):
                        pt = ps_bc.tile([128, 128], F32, tag="ptr", bufs=2)
                        nc.tensor.matmul(pt[:kn, :], lhsT=g[:, k0:k0 + kn],
                                         rhs=ident[:], start=True, stop=True)
                        if (i + k) % 2 == 0:
                            nc.vector.tensor_copy(
                                eT[k][:kn, i * 128:(i + 1) * 128], pt[:kn, :])
                        else:
                            nc.scalar.copy(
                                eT[k][:kn, i * 128:(i + 1) * 128], pt[:kn, :])
                for d in range(2):
                    for gt in range(3):
                        blk = _BLK[(d, gt)]
                        for ch in range(NCH // 2):
                            px = ps_bc.tile([128, 512], F32, tag="px", bufs=3)
                            for k, (k0, kn) in enumerate(KCH):
                                nc.tensor.matmul(
                                    px[:, :],
                                    lhsT=wih[:kn, d, k, gt * H:(gt + 1) * H],
                                    rhs=eT[k][:kn, ch * 512:(ch + 1) * 512],
                                    start=(k == 0), stop=(k == 2))
                            gch = half * (NCH // 2) + ch
                            dst = _sub(xp[:], gch * 32 * XPW + blk * BL,
                                       [[XPW, 32], [1, BL]])
                            src = _v(px, [[BL, 32], [1, BL]])
                            if (d * 3 + gt + ch) % 2 == 0:
                                nc.vector.tensor_scalar_add(
                                    dst, src, biasx[:, blk:blk + 1])
                            else:
                                nc.scalar.activation(
                                    dst, src, AF.Identity,
                                    bias=biasx[:, blk:blk + 1])

        if debug:
            nc.sync.dma_start(out=dbg_xp_d[:, :], in_=xp[:])

        # ---- phase D: scan ----
        nc.vector.memset(_sub(hs[:], 0, [[1, BL]]), 0.0)               # h_f(-1)
        nc.vector.memset(_sub(hs[:], HB0 + S * BL, [[1, BL]]), 0.0)    # h_b(S)
        with tc.tile_pool(name="ps_scan", bufs=1, space="PSUM") as ps_sc:
            for t in range(S):
                prz = ps_sc.tile([128, RZW], F32, tag="prz", bufs=3)
                pn = ps_sc.tile([128, NW], F32, tag="pn", bufs=3)
                hf = _sub(hs[:], t * BL, [[1, BL]])
                hb = _sub(hs[:], HB0 + (S - t) * BL, [[1, BL]])
                for d, hcur in ((0, hf), (1, hb)):
                    for gt in range(2):   # r, z
                        cb = _BLK[(d, gt)]
                        nc.tensor.matmul(prz[:, cb * BL:(cb + 1) * BL],
                                         lhsT=whh[:, d, gt * H:(gt + 1) * H],
                                         rhs=hcur, start=True, stop=True)
                for d, hcur in ((0, hf), (1, hb)):
                    nc.tensor.matmul(pn[:, d * BL:(d + 1) * BL],
                                     lhsT=whh[:, d, 2 * H:3 * H],
                                     rhs=hcur, start=True, stop=True)

                trz = work.tile([128, RZW], F32, tag="trz")
                xprz = _sub(xp[:], t * XPW,
                            [[(S - 1 - 2 * t) * XPW + 2 * BL, 2], [1, 2 * BL]])
                nc.vector.tensor_add(_v(trz, [[2 * BL, 2], [1, 2 * BL]]),
                                     _v(prz, [[2 * BL, 2], [1, 2 * BL]]),
                                     xprz)
                rw = work.tile([128, RZW], F32, tag="rw")
                nc.scalar.activation(rw[:], trz[:], AF.Sigmoid)
                # r blocks at 0,2 ; w blocks at 1,3 (w = 1-z, z pre-negated)
                r_v = _v(rw, [[2 * BL, 2], [1, BL]])
                w_v = _v(rw, [[2 * BL, 2], [1, BL]], off=BL)

                tn = work.tile([128, NW], F32, tag="tn")
                if zero_bhn:
                    nc.vector.tensor_tensor(_v(tn, [[BL, 2], [1, BL]]),
                                            _v(pn, [[BL, 2], [1, BL]]),
                                            r_v, op=OP.mult)
                else:
                    for d in range(2):
                        nc.vector.scalar_tensor_tensor(
                            _v(tn, [[1, BL]], off=d * BL),
                            _v(pn, [[1, BL]], off=d * BL),
                            bhn[:, d:d + 1],
                            _v(rw, [[1, BL]], off=2 * d * BL),
                            op0=OP.add, op1=OP.mult)
                t2 = work.tile([128, NW], F32, tag="t2")
                xpn = _sub(xp[:], t * XPW + RZW,
                           [[(S - 1 - 2 * t) * XPW + BL, 2], [1, BL]])
                nc.vector.tensor_add(_v(t2, [[BL, 2], [1, BL]]),
                                     _v(tn, [[BL, 2], [1, BL]]), xpn)
                n_t = work.tile([128, NW], F32, tag="n_t")
                nc.scalar.activation(n_t[:], t2[:], AF.Tanh)

                dlt = HB0 + (S - 2 * t) * BL
                hprev = _sub(hs[:], t * BL, [[dlt, 2], [1, BL]])
                dltw = HB0 + (S - 2 * t - 2) * BL
                hnew = _sub(hs[:], (t + 1) * BL, [[dltw, 2], [1, BL]])
                n_v = _v(n_t, [[BL, 2], [1, BL]])
                v_t = work.tile([128, NW], F32, tag="v_t")
                v_v = _v(v_t, [[BL, 2], [1, BL]])
                nc.gpsimd.tensor_tensor(v_v, n_v, hprev, op=OP.subtract)
                u_t = work.tile([128, NW], F32, tag="u_t")
                u_v = _v(u_t, [[BL, 2], [1, BL]])
                nc.gpsimd.tensor_tensor(u_v, w_v, v_v, op=OP.mult)
                nc.vector.tensor_tensor(hnew, u_v, hprev, op=OP.add)

        if debug:
            nc.sync.dma_start(out=dbg_hs_d[:, :], in_=hs[:])

        # ---- phases E/F/G ----
        with tc.tile_pool(name="ef", bufs=1) as ef, \
             tc.tile_pool(name="rp", bufs=1) as rp, \
             tc.tile_pool(name="ps_ef", bufs=1, space="PSUM") as ps_ef:
            # capsule u_hat [sb, 160]
            uh = ef.tile([128, NGRP * 160], F32)
            for c in range(NGRP):
                pu = ps_ef.tile([128, 160], F32, tag="pu", bufs=2)
                lhs_f = _sub(hs[:], (1 + 8 * c) * BL, [[1, 128]])
                lhs_b = _sub(hs[:], HB0 + 8 * c * BL, [[1, 128]])
                nc.tensor.matmul(pu[:], lhsT=lhs_f, rhs=wcap[:, 0, :],
                                 start=True, stop=False)
                nc.tensor.matmul(pu[:], lhsT=lhs_b, rhs=wcap[:, 1, :],
                                 start=False, stop=True)
                if c % 2 == 0:
                    nc.vector.tensor_copy(uh[:, c * 160:(c + 1) * 160], pu[:])
                else:
                    nc.scalar.copy(uh[:, c * 160:(c + 1) * 160], pu[:])

            if debug:
                nc.sync.dma_start(out=dbg_uh_d[:, :], in_=uh[:])

            # routing
            c_t = rp.tile([128, NGRP * NUM_CAP], F32, tag="c")   # [p, ch, cap]
            nc.vector.memset(c_t[:], 1.0 / NUM_CAP)
            bl_t = rp.tile([128, NGRP * NUM_CAP], F32, tag="bl")
            nc.gpsimd.memset(bl_t[:], 0.0)
            outputs = rp.tile([BL, 160], F32, tag="outs")
            tmp = rp.tile([128, NGRP * 160], F32, tag="tmp")

            for it in range(ROUTINGS):
                if it > 0:
                    # softmax over cap (free groups of 10)
                    mx = rp.tile([128, NGRP], F32, tag="mx", bufs=2)
                    nc.vector.tensor_reduce(
                        mx[:], _v(bl_t, [[NUM_CAP, NGRP], [1, NUM_CAP]]),
                        axis=AX.X, op=OP.max)
                    sb_t = rp.tile([128, NGRP * NUM_CAP], F32, tag="sb",
                                   bufs=2)
                    nc.vector.tensor_tensor(
                        _v(sb_t, [[NUM_CAP, NGRP], [1, NUM_CAP]]),
                        _v(bl_t, [[NUM_CAP, NGRP], [1, NUM_CAP]]),
                        _v(mx, [[1, NGRP], [0, NUM_CAP]]), op=OP.subtract)
                    nc.scalar.activation(sb_t[:], sb_t[:], AF.Exp)
                    sm = rp.tile([128, NGRP], F32, tag="sm", bufs=2)
                    nc.vector.tensor_reduce(
                        sm[:], _v(sb_t, [[NUM_CAP, NGRP], [1, NUM_CAP]]),
                        axis=AX.X, op=OP.add)
                    rc = rp.tile([128, NGRP], F32, tag="rc", bufs=2)
                    nc.vector.reciprocal(rc[:], sm[:])
                    nc.vector.tensor_tensor(
                        _v(c_t, [[NUM_CAP, NGRP], [1, NUM_CAP]]),
                        _v(sb_t, [[NUM_CAP, NGRP], [1, NUM_CAP]]),
                        _v(rc, [[1, NGRP], [0, NUM_CAP]]), op=OP.mult)

                # tmp = u_hat * c (c broadcast over dc), sum over s via matmul
                po = ps_ef.tile([BL, 160], F32, tag="po", bufs=2)
                for half in range(2):
                    lo = half * (NGRP // 2)
                    eng = nc.vector if half == 0 else nc.gpsimd
                    eng.tensor_tensor(
                        _sub(tmp[:], lo * 160,
                             [[160, NGRP // 2], [DIM_CAP, NUM_CAP],
                              [1, DIM_CAP]]),
                        _sub(uh[:], lo * 160,
                             [[160, NGRP // 2], [DIM_CAP, NUM_CAP],
                              [1, DIM_CAP]]),
                        _sub(c_t[:], lo * NUM_CAP,
                             [[NUM_CAP, NGRP // 2], [1, NUM_CAP],
                              [0, DIM_CAP]]),
                        op=OP.mult)
                for j in range(NGRP):
                    nc.tensor.matmul(po[:], lhsT=selB[:],
                                     rhs=tmp[:, j * 160:(j + 1) * 160],
                                     start=(j == 0), stop=(j == NGRP - 1))
                # squash
                sq = rp.tile([BL, 160], F32, tag="sq", bufs=2)
                nc.scalar.square(sq[:], po[:])
                ssum = rp.tile([BL, NUM_CAP], F32, tag="ssum", bufs=2)
                nc.vector.tensor_reduce(
                    ssum[:], _v(sq, [[DIM_CAP, NUM_CAP], [1, DIM_CAP]]),
                    axis=AX.X, op=OP.add)
                srt = rp.tile([BL, NUM_CAP], F32, tag="srt", bufs=2)
                nc.scalar.activation(srt[:], ssum[:], AF.Sqrt,
                                     bias=epst[:BL, 0:1])
                rs = rp.tile([BL, NUM_CAP], F32, tag="rs", bufs=2)
                nc.vector.reciprocal(rs[:], srt[:])
                nc.vector.tensor_tensor(
                    _v(outputs, [[DIM_CAP, NUM_CAP], [1, DIM_CAP]]),
                    _v(po, [[DIM_CAP, NUM_CAP], [1, DIM_CAP]]),
                    _v(rs, [[1, NUM_CAP], [0, DIM_CAP]]), op=OP.mult)

                if it < ROUTINGS - 1:
                    # broadcast outputs to all 128 partitions via selT matmul
                    pob = ps_ef.tile([128, 160], F32, tag="pob", bufs=1)
                    nc.tensor.matmul(pob[:], lhsT=selT[:], rhs=outputs[:],
                                     start=True, stop=True)
                    ob = rp.tile([128, 160], F32, tag="ob", bufs=2)
                    nc.scalar.copy(ob[:], pob[:])
                    # tmp = u_hat * ob (ob broadcast over chunks)
                    for half in range(2):
                        lo = half * (NGRP // 2)
                        eng = nc.vector if half == 0 else nc.gpsimd
                        eng.tensor_tensor(
                            _sub(tmp[:], lo * 160,
                                 [[160, NGRP // 2], [1, 160]]),
                            _sub(uh[:], lo * 160,
                                 [[160, NGRP // 2], [1, 160]]),
                            _v(ob, [[0, NGRP // 2], [1, 160]]),
                            op=OP.mult)
                    # du = sum over dc (innermost 16) -> [128, NGRP*NUM_CAP]
                    du = rp.tile([128, NGRP * NUM_CAP], F32, tag="du", bufs=2)
                    nc.vector.tensor_reduce(
                        _v(du, [[NUM_CAP, NGRP], [1, NUM_CAP]]),
                        _v(tmp, [[160, NGRP], [DIM_CAP, NUM_CAP],
                                 [1, DIM_CAP]]),
                        axis=AX.X, op=OP.add)
                    nc.vector.tensor_add(bl_t[:], bl_t[:], du[:])

            if debug:
                nc.sync.dma_start(out=dbg_bl_d[:, :], in_=bl_t[:])
                nc.sync.dma_start(out=dbg_o_d[:, :], in_=outputs[:])

            # final linear
            pt1 = ps_ef.tile([128, BL], F32, tag="pt1", bufs=1)
            nc.tensor.matmul(pt1[:, :], lhsT=outputs[:, 0:128],
                             rhs=ident[:BL, :BL], start=True, stop=True)
            pt2 = ps_ef.tile([32, BL], F32, tag="pt2", bufs=1)
            nc.tensor.matmul(pt2[:, :], lhsT=outputs[:, 128:160],
                             rhs=ident[:BL, :BL], start=True, stop=True)
            capsT = rp.tile([128, 2 * BL], F32, tag="capsT")
            nc.vector.tensor_copy(capsT[:, 0:BL], pt1[:])
            nc.vector.tensor_copy(capsT[:32, BL:2 * BL], pt2[:])
            pf = ps_ef.tile([2, BL], F32, tag="pf", bufs=1)
            nc.tensor.matmul(pf[:], lhsT=wlin[:, 0, :], rhs=capsT[:, 0:BL],
                             start=True, stop=False)
            nc.tensor.matmul(pf[:], lhsT=wlin[:32, 1, :],
                             rhs=capsT[:32, BL:2 * BL],
                             start=False, stop=True)
            outT = rp.tile([2, BL], F32, tag="outT")
            nc.scalar.activation(outT[:], pf[:], AF.Identity,
                                 bias=blin[:, 0:1])
            dst = bass.AP(tensor=out_d, offset=0, ap=[[1, 2], [2, BL]])
            nc.sync.dma_start(out=dst, in_=outT[:])

    return nc


_CACHE = {}


def _get_nc(zero_bhn):
    if zero_bhn not in _CACHE:
        nc = _build(zero_bhn)
        _split_waits(nc)   # HW-path legalization (CoreSim path builds its own)
        _CACHE[zero_bhn] = nc
    return _CACHE[zero_bhn]


def _host_inputs(x, emb, w_ih_f, w_hh_f, b_ih_f, b_hh_f,
                 w_ih_b, w_hh_b, b_ih_b, b_hh_b, W_cap, W_lin, b_lin):
    """Build the per-core input maps (everything but xidx is shared)."""
    f32 = np.float32
    neg = np.ones((G3,), f32)
    neg[H:2 * H] = -1.0        # negate z gate (sigmoid -> 1-z)

    wih = np.stack([(w_ih_f.T * neg).astype(f32), (w_ih_b.T * neg).astype(f32)])
    whh = np.stack([(w_hh_f.T * neg).astype(f32), (w_hh_b.T * neg).astype(f32)])

    biasx = np.zeros((128, 6), f32)
    for d, (bi, bh) in enumerate([(b_ih_f, b_hh_f), (b_ih_b, b_hh_b)]):
        biasx[:, _BLK[(d, 0)]] = (bi[0:H] + bh[0:H])
        biasx[:, _BLK[(d, 1)]] = -(bi[H:2 * H] + bh[H:2 * H])
        biasx[:, _BLK[(d, 2)]] = bi[2 * H:3 * H]
    bhn = np.zeros((128, 2), f32)
    bhn[:, 0] = b_hh_f[2 * H:3 * H]
    bhn[:, 1] = b_hh_b[2 * H:3 * H]
    zero_bhn = bool(np.all(bhn == 0.0))

    wcap = np.stack([W_cap[0:H, :].astype(f32), W_cap[H:2 * H, :].astype(f32)])
    selB = (np.arange(128)[:, None] % BL == np.arange(BL)[None, :]).astype(f32)
    selT = selB.T.copy()
    ident = np.eye(128, dtype=f32)

    shared = dict(emb=np.ascontiguousarray(emb, f32), wih=wih, whh=whh,
                  biasx=biasx, bhn=bhn, wcap=wcap,
                  wlin=np.ascontiguousarray(W_lin, f32),
                  blin=np.ascontiguousarray(b_lin, f32).reshape(2, 1),
                  selB=selB, selT=selT, ident=ident)

    in_maps = []
    for c in range(NCORES):
        xl = np.asarray(x[c * BL:(c + 1) * BL, :])          # [BL, S]
        tok = xl.T.reshape(-1).astype(np.int32)             # s-major [NTOK]
        xidx = np.ascontiguousarray(tok.reshape(NGRP, 128).T)  # [128, NGRP]
        in_maps.append(dict(shared, xidx=xidx))
    return in_maps, zero_bhn


def kernel(**inputs):
    in_maps, zero_bhn = _host_inputs(**{k: np.asarray(v) for k, v in
                                        inputs.items()})
    nc = _get_nc(zero_bhn)
    res = run_bass_kernel_spmd(nc, in_maps, list(range(NCORES)))
    return np.concatenate([res.results[c]["out"] for c in range(NCORES)],
                          axis=0)


def _install_ntff_hook():
    """Shim the missing antenv.axon_hooks so trace=True works under axon."""
    import sys, types
    if "antenv.axon_hooks" in sys.modules:
        return
    mod = types.ModuleType("antenv.axon_hooks")
    _h = [None]
    mod.set_axon_ntff_profile_hook = lambda h: _h.__setitem__(0, h)
    mod.get_axon_ntff_profile_hook = lambda: _h[0]
    sys.modules["antenv.axon_hooks"] = mod
    import antenv
    antenv.axon_hooks = mod
    from trn_agent_boot.trn_boot import _ntff_profile_via_ctypes
    mod.set_axon_ntff_profile_hook(
        _ntff_profile_via_ctypes("/opt/axon/libaxon_pjrt.so"))


def kernel_profiled(**inputs):
    """Same as kernel() but with NTFF tracing; returns (out, result_obj)."""
    _install_ntff_hook()
    in_maps, zero_bhn = _host_inputs(**{k: np.asarray(v) for k, v in
                                        inputs.items()})
    nc = _get_nc(zero_bhn)
    res = run_bass_kernel_spmd(nc, in_maps, list(range(NCORES)), trace=True)
    out = np.concatenate([res.results[c]["out"] for c in range(NCORES)],
                         axis=0)
    return out, res


# revision 29
# speedup vs baseline: 1.5804x; 1.5804x over previous
"""Trainium2 Bass kernel for nn_CapRNNModelHelper (bi-GRU + capsule routing).

Sharding: data-parallel over batch across 8 cores (16 batch rows per core).
Everything else (embedding table, GRU weights, capsule weights) replicated.

Per-core pipeline (v2, bf16 matmul operands, f32 accumulation):
  1. indirect-DMA gather of embedding rows (token order s-major), cast bf16
  2. PE-transpose (plain matmul vs identity) -> e.T  [300, ntok] bf16
  3. x_proj matmuls (bf16) -> xp_rz (bf16) + xp_n (f32), biases folded,
     z blocks negated so sigmoid gives w = 1-z directly
  4. 256-step fused bidirectional GRU scan: per step the xp_rz slice is
     PSUM-accumulated via an identity matmul, gates matmul on top (bf16
     weights, bf16 h mirror), sigmoid/tanh on ACT, update on DVE+GpSimd
  5. capsule matmul (bf16) -> u_hat [sb, 160] f32
  6. 5-iter dynamic routing (selector matmuls for sequence reductions)
  7. final linear -> out [16, 2]
"""

import numpy as np
from contextlib import ExitStack

import concourse.bass as bass
import concourse.tile as tile
from concourse import mybir
from concourse.bass import IndirectOffsetOnAxis
from concourse.bass_utils import run_bass_kernel_spmd
from concourse.tile_rust import add_dep_helper

F32 = mybir.dt.float32
BF16 = mybir.dt.float16
I32 = mybir.dt.int32
AF = mybir.ActivationFunctionType
OP = mybir.AluOpType
AX = mybir.AxisListType

VOCAB, D_W, H, S, B = 50000, 300, 128, 256, 128
NUM_CAP, DIM_CAP, ROUTINGS, EPS = 10, 16, 5, 1e-7
NCORES = 8
BL = B // NCORES          # 16 batch rows per core
NTOK = S * BL             # 4096 tokens per core
NGRP = NTOK // 128        # 32 gather groups of 128 tokens
NCH = NTOK // 512         # 8 x_proj chunks of 512 tokens
KCH = [(0, 128), (128, 128), (256, 44)]   # D_W split
G3 = 3 * H                # 384

RZW = 4 * BL              # 64   per-step rz width [rf zf rb zb]
NW = 2 * BL               # 32   per-step n width [nf nb]
# block index for (dir d, gate g): rz blocks 0..3, n blocks 0..1
_BLKRZ = {(0, 0): 0, (0, 1): 1, (1, 0): 2, (1, 1): 3}


def _sub(base, off, dims):
    """Manual AP: base is a [128, X] AP; append free dims after partition."""
    return bass.AP(tensor=base.tensor, offset=base.offset + off,
                   ap=[base.ap[0]] + dims)


def _v(t, dims, off=0):
    return bass.AP(tensor=t.tensor, offset=t.offset + off,
                   ap=[t.ap[0]] + dims)


def _split_waits(nc, cap=1):
    """Hoist excess sync waits onto standalone event-semaphore ops.

    The walrus build on this stack accepts only `cap` sync-wait commands
    per ISA instruction; Tile can attach several. Event-semaphore ops on
    the same engine execute in queue order, so hoisting preserves
    semantics.
    """
    n = 0
    for fn in nc.m.functions:
        for bb in fn.blocks:
            out = []
            for ins in bb.instructions:
                si = ins.sync_info
                if si is not None and len(si.on_wait) > cap:
                    waits = list(si.on_wait)
                    keep = waits[len(waits) - cap:] if cap else []
                    for w in waits[:len(waits) - cap] if cap else waits:
                        n += 1
                        out.append(mybir.InstEventSemaphore(
                            name=f"wsplit-{n}", engine=ins.engine,
                            ins=[], outs=[],
                            sync_info=mybir.SyncInfo(on_wait=[w],
                                                     on_update=[])))
                    ins.sync_info = mybir.SyncInfo(
                        on_wait=keep, on_update=list(si.on_update))
                out.append(ins)
            bb.instructions = out
    return n


def _build(zero_bhn: bool, debug: bool = False):
    nc = bass.Bass()
    if debug:
        dbg_hs_d = nc.declare_dram_parameter("dbg_hs", [128, 2 * (S + 1) * BL],
                                             F32, True)
        dbg_uh_d = nc.declare_dram_parameter("dbg_uh", [128, NGRP * 160], F32,
                                             True)
        dbg_bl_d = nc.declare_dram_parameter("dbg_bl", [128, NGRP * NUM_CAP],
                                             F32, True)
        dbg_o_d = nc.declare_dram_parameter("dbg_o", [BL, 160], F32, True)

    xidx_d = nc.declare_dram_parameter("xidx", [128, NGRP], I32, False)
    emb_d = nc.declare_dram_parameter("emb", [VOCAB, D_W], F32, False)
    wih_d = nc.declare_dram_parameter("wih", [2, D_W, G3], BF16, False)
    whh_d = nc.declare_dram_parameter("whh", [2, H, G3], BF16, False)
    biasx_d = nc.declare_dram_parameter("biasx", [128, 6], F32, False)
    bhn_d = nc.declare_dram_parameter("bhn", [128, 2], F32, False)
    wcap_d = nc.declare_dram_parameter("wcap", [2, H, 160], BF16, False)
    wlin_d = nc.declare_dram_parameter("wlin", [160, 2], F32, False)
    blin_d = nc.declare_dram_parameter("blin", [2, 1], F32, False)
    selB_d = nc.declare_dram_parameter("selB", [128, BL], F32, False)
    selT_d = nc.declare_dram_parameter("selT", [BL, 128], F32, False)
    ident_d = nc.declare_dram_parameter("ident", [128, 128], F32, False)
    out_d = nc.declare_dram_parameter("out", [BL, 2], F32, True)

    with tile.TileContext(nc) as tc, ExitStack() as ctx:
        const = ctx.enter_context(tc.tile_pool(name="const", bufs=1))
        bigxp = ctx.enter_context(tc.tile_pool(name="bigxp", bufs=1))
        bighs = ctx.enter_context(tc.tile_pool(name="bighs", bufs=1))
        work = ctx.enter_context(tc.tile_pool(name="work", bufs=3))

        # ---- constants to SBUF ----
        xidx = const.tile([128, NGRP], I32)
        nc.sync.dma_start(out=xidx[:], in_=xidx_d[:, :])
        whh = const.tile([128, 2, G3], BF16)
        for d in range(2):
            nc.sync.dma_start(out=whh[:, d, :], in_=whh_d[d, :, :])
        biasx = const.tile([128, 6], F32)
        nc.sync.dma_start(out=biasx[:], in_=biasx_d[:, :])
        bhn = const.tile([128, 2], F32)
        nc.sync.dma_start(out=bhn[:], in_=bhn_d[:, :])
        wcap = const.tile([128, 2, 160], BF16)
        for k in range(2):
            nc.sync.dma_start(out=wcap[:, k, :], in_=wcap_d[k, :, :])
        wlin = const.tile([128, 2, 2], F32)        # chunk0 [:128], chunk1 [:32]
        nc.sync.dma_start(out=wlin[:, 0, :], in_=wlin_d[0:128, :])
        nc.sync.dma_start(out=wlin[:32, 1, :], in_=wlin_d[128:160, :])
        blin = const.tile([2, 1], F32)
        nc.sync.dma_start(out=blin[:], in_=blin_d[:, :])
        selB = const.tile([128, BL], F32)
        nc.sync.dma_start(out=selB[:], in_=selB_d[:, :])
        selT = const.tile([BL, 128], F32)
        nc.sync.dma_start(out=selT[:], in_=selT_d[:, :])
        ident = const.tile([128, 128], F32)
        nc.sync.dma_start(out=ident[:], in_=ident_d[:, :])
        identb = const.tile([128, 128], BF16)
        nc.scalar.copy(identb[:], ident[:])
        epst = const.tile([128, 1], F32)
        nc.vector.memset(epst[:], EPS)

        xprz = bigxp.tile([128, S * RZW], BF16)     # 32.8 KB/part
        xpn = bigxp.tile([128, S * NW], F32)        # 32.8 KB/part
        HB0 = (S + 1) * BL
        hs = bighs.tile([128, 2 * (S + 1) * BL], F32)    # 32.9 KB/part
        hbf = bighs.tile([128, 2 * (S + 1) * BL], BF16)  # 16.4 KB/part

        # ---- phases B+C: gather + cast + transpose + x_proj, 2 half passes --
        HTOK = NTOK // 2
        with tc.tile_pool(name="bc", bufs=1) as bc, \
             tc.tile_pool(name="gat", bufs=4) as gat, \
             tc.tile_pool(name="ps_bc", bufs=1, space="PSUM") as ps_bc:
            wih = bc.tile([128, 2, 3, G3], BF16)   # [kpart, dir, kchunk, gcol]
            for d in range(2):
                for k, (k0, kn) in enumerate(KCH):
                    nc.sync.dma_start(out=wih[:kn, d, k, :],
                                      in_=wih_d[d, k0:k0 + kn, :])
            for half in range(2):
                eT = [bc.tile([128, HTOK], BF16, name=f"eT{k}", tag=f"eT{k}")
                      for k in range(3)]
                for i in range(NGRP // 2):
                    ig = half * (NGRP // 2) + i
                    g = gat.tile([128, D_W], F32, name="g", tag="g")
                    nc.gpsimd.indirect_dma_start(
                        out=g[:], out_offset=None,
                        in_=emb_d[:, :],
                        in_offset=IndirectOffsetOnAxis(ap=xidx[:, ig:ig + 1],
                                                       axis=0))
                    gb = gat.tile([128, D_W], BF16, name="gb", tag="gb")
                    nc.gpsimd.tensor_copy(gb[:], g[:])
                    for k, (k0, kn) in enumerate(KCH):
                        pt = ps_bc.tile([128, 128], F32, tag="ptr", bufs=2)
                        nc.tensor.matmul(pt[:kn, :], lhsT=gb[:, k0:k0 + kn],
                                         rhs=identb[:], start=True, stop=True)
                        if (i + k) % 2 == 0:
                            nc.vector.tensor_copy(
                                eT[k][:kn, i * 128:(i + 1) * 128], pt[:kn, :])
                        else:
                            nc.scalar.copy(
                                eT[k][:kn, i * 128:(i + 1) * 128], pt[:kn, :])
                for d in range(2):
                    for gt in range(3):
                        for ch in range(NCH // 2):
                            px = ps_bc.tile([128, 512], F32, tag="px", bufs=3)
                            for k, (k0, kn) in enumerate(KCH):
                                nc.tensor.matmul(
                                    px[:, :],
                                    lhsT=wih[:kn, d, k, gt * H:(gt + 1) * H],
                                    rhs=eT[k][:kn, ch * 512:(ch + 1) * 512],
                                    start=(k == 0), stop=(k == 2))
                            gch = half * (NCH // 2) + ch
                            src = _v(px, [[BL, 32], [1, BL]])
                            if gt < 2:
                                blk = _BLKRZ[(d, gt)]
                                dst = _sub(xprz[:], gch * 32 * RZW + blk * BL,
                                           [[RZW, 32], [1, BL]])
                                bcol = blk
                            else:
                                dst = _sub(xpn[:], gch * 32 * NW + d * BL,
                                           [[NW, 32], [1, BL]])
                                bcol = 4 + d
                            if (d * 3 + gt + ch) % 2 == 0:
                                nc.vector.tensor_scalar_add(
                                    dst, src, biasx[:, bcol:bcol + 1])
                            else:
                                nc.scalar.activation(
                                    dst, src, AF.Identity,
                                    bias=biasx[:, bcol:bcol + 1])

        # ---- phase D: scan ----
        nc.vector.memset(_sub(hs[:], 0, [[1, BL]]), 0.0)               # h_f(-1)
        nc.vector.memset(_sub(hs[:], HB0 + S * BL, [[1, BL]]), 0.0)    # h_b(S)
        nc.gpsimd.memset(_sub(hbf[:], 0, [[1, BL]]), 0.0)
        nc.gpsimd.memset(_sub(hbf[:], HB0 + S * BL, [[1, BL]]), 0.0)
        with tc.tile_pool(name="ps_scan", bufs=1, space="PSUM") as ps_sc:
            for t in range(S):
                prz = ps_sc.tile([128, RZW], F32, tag="prz", bufs=3)
                pn = ps_sc.tile([128, NW], F32, tag="pn", bufs=3)
                hfb = _sub(hbf[:], t * BL, [[1, BL]])
                hbb = _sub(hbf[:], HB0 + (S - t) * BL, [[1, BL]])
                # xp_rz preload via identity matmul (f at t, b at S-1-t)
                xrz = _sub(xprz[:], t * RZW,
                           [[(S - 1 - 2 * t) * RZW + 2 * BL, 2], [1, 2 * BL]])
                prev = nc.tensor.matmul(prz[:], lhsT=identb[:], rhs=xrz,
                                        start=True, stop=False)
                for j, (d, hcur) in enumerate(((0, hfb), (0, hfb),
                                               (1, hbb), (1, hbb))):
                    gt = j % 2
                    cb = _BLKRZ[(d, gt)]
                    gm = nc.tensor.matmul(
                        prz[:, cb * BL:(cb + 1) * BL],
                        lhsT=whh[:, d, gt * H:(gt + 1) * H],
                        rhs=hcur, start=False, stop=(j == 3))
                    add_dep_helper(gm.ins, prev.ins, sync=False,
                                   reason="psum accum order")
                    prev = gm
                for d, hcur in ((0, hfb), (1, hbb)):
                    nc.tensor.matmul(pn[:, d * BL:(d + 1) * BL],
                                     lhsT=whh[:, d, 2 * H:3 * H],
                                     rhs=hcur, start=True, stop=True)

                rw = work.tile([128, RZW], F32, tag="rw")
                nc.scalar.activation(rw[:], prz[:], AF.Sigmoid)
                # r blocks at 0,2 ; w blocks at 1,3 (w = 1-z, z pre-negated)
                r_v = _v(rw, [[2 * BL, 2], [1, BL]])
                w_v = _v(rw, [[2 * BL, 2], [1, BL]], off=BL)

                tn = work.tile([128, NW], F32, tag="tn")
                if zero_bhn:
                    nc.vector.tensor_tensor(_v(tn, [[BL, 2], [1, BL]]),
                                            _v(pn, [[BL, 2], [1, BL]]),
                                            r_v, op=OP.mult)
                else:
                    for d in range(2):
                        nc.vector.scalar_tensor_tensor(
                            _v(tn, [[1, BL]], off=d * BL),
                            _v(pn, [[1, BL]], off=d * BL),
                            bhn[:, d:d + 1],
                            _v(rw, [[1, BL]], off=2 * d * BL),
                            op0=OP.add, op1=OP.mult)
                t2 = work.tile([128, NW], F32, tag="t2")
                xn = _sub(xpn[:], t * NW,
                          [[(S - 1 - 2 * t) * NW + BL, 2], [1, BL]])
                nc.vector.tensor_add(_v(t2, [[BL, 2], [1, BL]]),
                                     _v(tn, [[BL, 2], [1, BL]]), xn)
                n_t = work.tile([128, NW], F32, tag="n_t")
                nc.scalar.activation(n_t[:], t2[:], AF.Tanh)

                dlt = HB0 + (S - 2 * t) * BL
                hprev = _sub(hs[:], t * BL, [[dlt, 2], [1, BL]])
                dltw = HB0 + (S - 2 * t - 2) * BL
                hnew = _sub(hs[:], (t + 1) * BL, [[dltw, 2], [1, BL]])
                hnewb = _sub(hbf[:], (t + 1) * BL, [[dltw, 2], [1, BL]])
                n_v = _v(n_t, [[BL, 2], [1, BL]])
                v_t = work.tile([128, NW], F32, tag="v_t")
                v_v = _v(v_t, [[BL, 2], [1, BL]])
                nc.gpsimd.tensor_tensor(v_v, n_v, hprev, op=OP.subtract)
                u_t = work.tile([128, NW], F32, tag="u_t")
                u_v = _v(u_t, [[BL, 2], [1, BL]])
                nc.vector.tensor_tensor(u_v, w_v, v_v, op=OP.mult)
                nc.vector.tensor_tensor(hnew, u_v, hprev, op=OP.add)
                nc.gpsimd.tensor_tensor(hnewb, u_v, hprev, op=OP.add)

        if debug:
            nc.sync.dma_start(out=dbg_hs_d[:, :], in_=hs[:])

        # ---- phases E/F/G ----
        with tc.tile_pool(name="ef", bufs=1) as ef, \
             tc.tile_pool(name="rp", bufs=1) as rp, \
             tc.tile_pool(name="ps_ef", bufs=1, space="PSUM") as ps_ef:
            # capsule u_hat [sb, 160]
            uh = ef.tile([128, NGRP * 160], F32)
            for c in range(NGRP):
                pu = ps_ef.tile([128, 160], F32, tag="pu", bufs=2)
                lhs_f = _sub(hbf[:], (1 + 8 * c) * BL, [[1, 128]])
                lhs_b = _sub(hbf[:], HB0 + 8 * c * BL, [[1, 128]])
                nc.tensor.matmul(pu[:], lhsT=lhs_f, rhs=wcap[:, 0, :],
                                 start=True, stop=False)
                nc.tensor.matmul(pu[:], lhsT=lhs_b, rhs=wcap[:, 1, :],
                                 start=False, stop=True)
                if c % 2 == 0:
                    nc.vector.tensor_copy(uh[:, c * 160:(c + 1) * 160], pu[:])
                else:
                    nc.scalar.copy(uh[:, c * 160:(c + 1) * 160], pu[:])

            if debug:
                nc.sync.dma_start(out=dbg_uh_d[:, :], in_=uh[:])

            # routing
            c_t = rp.tile([128, NGRP * NUM_CAP], F32, tag="c")   # [p, ch, cap]
            nc.vector.memset(c_t[:], 1.0 / NUM_CAP)
            bl_t = rp.tile([128, NGRP * NUM_CAP], F32, tag="bl")
            nc.gpsimd.memset(bl_t[:], 0.0)
            outputs = rp.tile([BL, 160], F32, tag="outs")
            tmp = rp.tile([128, NGRP * 160], F32, tag="tmp")

            for it in range(ROUTINGS):
                if it > 0:
                    # softmax over cap (free groups of 10)
                    mx = rp.tile([128, NGRP], F32, tag="mx", bufs=2)
                    nc.vector.tensor_reduce(
                        mx[:], _v(bl_t, [[NUM_CAP, NGRP], [1, NUM_CAP]]),
                        axis=AX.X, op=OP.max)
                    sb_t = rp.tile([128, NGRP * NUM_CAP], F32, tag="sb",
                                   bufs=2)
                    nc.vector.tensor_tensor(
                        _v(sb_t, [[NUM_CAP, NGRP], [1, NUM_CAP]]),
                        _v(bl_t, [[NUM_CAP, NGRP], [1, NUM_CAP]]),
                        _v(mx, [[1, NGRP], [0, NUM_CAP]]), op=OP.subtract)
                    nc.scalar.activation(sb_t[:], sb_t[:], AF.Exp)
                    sm = rp.tile([128, NGRP], F32, tag="sm", bufs=2)
                    nc.vector.tensor_reduce(
                        sm[:], _v(sb_t, [[NUM_CAP, NGRP], [1, NUM_CAP]]),
                        axis=AX.X, op=OP.add)
                    rc = rp.tile([128, NGRP], F32, tag="rc", bufs=2)
                    nc.vector.reciprocal(rc[:], sm[:])
                    nc.vector.tensor_tensor(
                        _v(c_t, [[NUM_CAP, NGRP], [1, NUM_CAP]]),
                        _v(sb_t, [[NUM_CAP, NGRP], [1, NUM_CAP]]),
                        _v(rc, [[1, NGRP], [0, NUM_CAP]]), op=OP.mult)

                # tmp = u_hat * c (c broadcast over dc), sum over s via matmul
                po = ps_ef.tile([BL, 160], F32, tag="po", bufs=2)
                for half in range(2):
                    lo = half * (NGRP // 2)
                    eng = nc.vector if half == 0 else nc.gpsimd
                    eng.tensor_tensor(
                        _sub(tmp[:], lo * 160,
                             [[160, NGRP // 2], [DIM_CAP, NUM_CAP],
                              [1, DIM_CAP]]),
                        _sub(uh[:], lo * 160,
                             [[160, NGRP // 2], [DIM_CAP, NUM_CAP],
                              [1, DIM_CAP]]),
                        _sub(c_t[:], lo * NUM_CAP,
                             [[NUM_CAP, NGRP // 2], [1, NUM_CAP],
                              [0, DIM_CAP]]),
                        op=OP.mult)
                for j in range(NGRP):
                    nc.tensor.matmul(po[:], lhsT=selB[:],
                                     rhs=tmp[:, j * 160:(j + 1) * 160],
                                     start=(j == 0), stop=(j == NGRP - 1))
                # squash
                sq = rp.tile([BL, 160], F32, tag="sq", bufs=2)
                nc.scalar.square(sq[:], po[:])
                ssum = rp.tile([BL, NUM_CAP], F32, tag="ssum", bufs=2)
                nc.vector.tensor_reduce(
                    ssum[:], _v(sq, [[DIM_CAP, NUM_CAP], [1, DIM_CAP]]),
                    axis=AX.X, op=OP.add)
                srt = rp.tile([BL, NUM_CAP], F32, tag="srt", bufs=2)
                nc.scalar.activation(srt[:], ssum[:], AF.Sqrt,
                                     bias=epst[:BL, 0:1])
                rs = rp.tile([BL, NUM_CAP], F32, tag="rs", bufs=2)
                nc.vector.reciprocal(rs[:], srt[:])
                nc.vector.tensor_tensor(
                    _v(outputs, [[DIM_CAP, NUM_CAP], [1, DIM_CAP]]),
                    _v(po, [[DIM_CAP, NUM_CAP], [1, DIM_CAP]]),
                    _v(rs, [[1, NUM_CAP], [0, DIM_CAP]]), op=OP.mult)

                if it < ROUTINGS - 1:
                    # broadcast outputs to all 128 partitions via selT matmul
                    pob = ps_ef.tile([128, 160], F32, tag="pob", bufs=1)
                    nc.tensor.matmul(pob[:], lhsT=selT[:], rhs=outputs[:],
                                     start=True, stop=True)
                    ob = rp.tile([128, 160], F32, tag="ob", bufs=2)
                    nc.scalar.copy(ob[:], pob[:])
                    # tmp = u_hat * ob (ob broadcast over chunks)
                    for half in range(2):
                        lo = half * (NGRP // 2)
                        eng = nc.vector if half == 0 else nc.gpsimd
                        eng.tensor_tensor(
                            _sub(tmp[:], lo * 160,
                                 [[160, NGRP // 2], [1, 160]]),
                            _sub(uh[:], lo * 160,
                                 [[160, NGRP // 2], [1, 160]]),
                            _v(ob, [[0, NGRP // 2], [1, 160]]),
                            op=OP.mult)
                    # du = sum over dc (innermost 16) -> [128, NGRP*NUM_CAP]
                    du = rp.tile([128, NGRP * NUM_CAP], F32, tag="du", bufs=2)
                    nc.vector.tensor_reduce(
                        _v(du, [[NUM_CAP, NGRP], [1, NUM_CAP]]),
                        _v(tmp, [[160, NGRP], [DIM_CAP, NUM_CAP],
                                 [1, DIM_CAP]]),
                        axis=AX.X, op=OP.add)
                    nc.vector.tensor_add(bl_t[:], bl_t[:], du[:])

            if debug:
                nc.sync.dma_start(out=dbg_bl_d[:, :], in_=bl_t[:])
                nc.sync.dma_start(out=dbg_o_d[:, :], in_=outputs[:])

            # final linear
            pt1 = ps_ef.tile([128, BL], F32, tag="pt1", bufs=1)
            nc.tensor.matmul(pt1[:, :], lhsT=outputs[:, 0:128],
                             rhs=ident[:BL, :BL], start=True, stop=True)
            pt2 = ps_ef.tile([32, BL], F32, tag="pt2", bufs=1)
            nc.tensor.matmul(pt2[:, :], lhsT=outputs[:, 128:160],
                             rhs=ident[:BL, :BL], start=True, stop=True)
            capsT = rp.tile([128, 2 * BL], F32, tag="capsT")
            nc.vector.tensor_copy(capsT[:, 0:BL], pt1[:])
            nc.vector.tensor_copy(capsT[:32, BL:2 * BL], pt2[:])
            pf = ps_ef.tile([2, BL], F32, tag="pf", bufs=1)
            nc.tensor.matmul(pf[:], lhsT=wlin[:, 0, :], rhs=capsT[:, 0:BL],
                             start=True, stop=False)
            nc.tensor.matmul(pf[:], lhsT=wlin[:32, 1, :],
                             rhs=capsT[:32, BL:2 * BL],
                             start=False, stop=True)
            outT = rp.tile([2, BL], F32, tag="outT")
            nc.scalar.activation(outT[:], pf[:], AF.Identity,
                                 bias=blin[:, 0:1])
            dst = bass.AP(tensor=out_d, offset=0, ap=[[1, 2], [2, BL]])
            nc.sync.dma_start(out=dst, in_=outT[:])

    return nc


_CACHE = {}


def _get_nc(zero_bhn):
    if zero_bhn not in _CACHE:
        nc = _build(zero_bhn)
        _split_waits(nc)   # HW-path legalization (CoreSim path builds its own)
        _CACHE[zero_bhn] = nc
    return _CACHE[zero_bhn]


def _host_inputs(x, emb, w_ih_f, w_hh_f, b_ih_f, b_hh_f,
                 w_ih_b, w_hh_b, b_ih_b, b_hh_b, W_cap, W_lin, b_lin):
    """Build the per-core input maps (everything but xidx is shared)."""
    import ml_dtypes
    f32 = np.float32
    bf16 = np.float16
    neg = np.ones((G3,), f32)
    neg[H:2 * H] = -1.0        # negate z gate (sigmoid -> 1-z)

    wih = np.stack([(w_ih_f.T * neg).astype(bf16), (w_ih_b.T * neg).astype(bf16)])
    whh = np.stack([(w_hh_f.T * neg).astype(bf16), (w_hh_b.T * neg).astype(bf16)])

    biasx = np.zeros((128, 6), f32)
    for d, (bi, bh) in enumerate([(b_ih_f, b_hh_f), (b_ih_b, b_hh_b)]):
        biasx[:, _BLKRZ[(d, 0)]] = (bi[0:H] + bh[0:H])
        biasx[:, _BLKRZ[(d, 1)]] = -(bi[H:2 * H] + bh[H:2 * H])
        biasx[:, 4 + d] = bi[2 * H:3 * H]
    bhn = np.zeros((128, 2), f32)
    bhn[:, 0] = b_hh_f[2 * H:3 * H]
    bhn[:, 1] = b_hh_b[2 * H:3 * H]
    zero_bhn = bool(np.all(bhn == 0.0))

    wcap = np.stack([W_cap[0:H, :].astype(bf16), W_cap[H:2 * H, :].astype(bf16)])
    selB = (np.arange(128)[:, None] % BL == np.arange(BL)[None, :]).astype(f32)
    selT = selB.T.copy()
    ident = np.eye(128, dtype=f32)

    shared = dict(emb=np.ascontiguousarray(emb, f32), wih=wih, whh=whh,
                  biasx=biasx, bhn=bhn, wcap=wcap,
                  wlin=np.ascontiguousarray(W_lin, f32),
                  blin=np.ascontiguousarray(b_lin, f32).reshape(2, 1),
                  selB=selB, selT=selT, ident=ident)

    in_maps = []
    for c in range(NCORES):
        xl = np.asarray(x[c * BL:(c + 1) * BL, :])          # [BL, S]
        tok = xl.T.reshape(-1).astype(np.int32)             # s-major [NTOK]
        xidx = np.ascontiguousarray(tok.reshape(NGRP, 128).T)  # [128, NGRP]
        in_maps.append(dict(shared, xidx=xidx))
    return in_maps, zero_bhn


def kernel(**inputs):
    in_maps, zero_bhn = _host_inputs(**{k: np.asarray(v) for k, v in
                                        inputs.items()})
    nc = _get_nc(zero_bhn)
    res = run_bass_kernel_spmd(nc, in_maps, list(range(NCORES)))
    return np.concatenate([res.results[c]["out"] for c in range(NCORES)],
                          axis=0)


def _install_ntff_hook():
    """Shim the missing antenv.axon_hooks so trace=True works under axon."""
    import sys, types
    if "antenv.axon_hooks" in sys.modules:
        return
    mod = types.ModuleType("antenv.axon_hooks")
    _h = [None]
    mod.set_axon_ntff_profile_hook = lambda h: _h.__setitem__(0, h)
    mod.get_axon_ntff_profile_hook = lambda: _h[0]
    sys.modules["antenv.axon_hooks"] = mod
    import antenv
    antenv.axon_hooks = mod
    from trn_agent_boot.trn_boot import _ntff_profile_via_ctypes
    mod.set_axon_ntff_profile_hook(
        _ntff_profile_via_ctypes("/opt/axon/libaxon_pjrt.so"))


def kernel_profiled(**inputs):
    """Same as kernel() but with NTFF tracing; returns (out, result_obj)."""
    _install_ntff_hook()
    in_maps, zero_bhn = _host_inputs(**{k: np.asarray(v) for k, v in
                                        inputs.items()})
    nc = _get_nc(zero_bhn)
    res = run_bass_kernel_spmd(nc, in_maps, list(range(NCORES)), trace=True)
    out = np.concatenate([res.results[c]["out"] for c in range(NCORES)],
                         axis=0)
    return out, res


# revision 30
# speedup vs baseline: 1.7763x; 1.1240x over previous
"""Trainium2 Bass kernel for nn_CapRNNModelHelper (bi-GRU + capsule routing).

Sharding: data-parallel over batch across 8 cores (16 batch rows per core).
Everything else (embedding table, GRU weights, capsule weights) replicated.

Per-core pipeline (v2, bf16 matmul operands, f32 accumulation):
  1. indirect-DMA gather of embedding rows (token order s-major), cast bf16
  2. PE-transpose (plain matmul vs identity) -> e.T  [300, ntok] bf16
  3. x_proj matmuls (bf16) -> xp_rz (bf16) + xp_n (f32), biases folded,
     z blocks negated so sigmoid gives w = 1-z directly
  4. 256-step fused bidirectional GRU scan: per step the xp_rz slice is
     PSUM-accumulated via an identity matmul, gates matmul on top (bf16
     weights, bf16 h mirror), sigmoid/tanh on ACT, update on DVE+GpSimd
  5. capsule matmul (bf16) -> u_hat [sb, 160] f32
  6. 5-iter dynamic routing (selector matmuls for sequence reductions)
  7. final linear -> out [16, 2]
"""

import numpy as np
from contextlib import ExitStack

import concourse.bass as bass
import concourse.tile as tile
from concourse import mybir
from concourse.bass import IndirectOffsetOnAxis
from concourse.bass_utils import run_bass_kernel_spmd
from concourse.tile_rust import add_dep_helper

F32 = mybir.dt.float32
BF16 = mybir.dt.float16
I32 = mybir.dt.int32
AF = mybir.ActivationFunctionType
OP = mybir.AluOpType
AX = mybir.AxisListType

VOCAB, D_W, H, S, B = 50000, 300, 128, 256, 128
NUM_CAP, DIM_CAP, ROUTINGS, EPS = 10, 16, 5, 1e-7
NCORES = 8
BL = B // NCORES          # 16 batch rows per core
NTOK = S * BL             # 4096 tokens per core
NGRP = NTOK // 128        # 32 gather groups of 128 tokens
NCH = NTOK // 512         # 8 x_proj chunks of 512 tokens
KCH = [(0, 128), (128, 128), (256, 44)]   # D_W split
G3 = 3 * H                # 384

RZW = 4 * BL              # 64   per-step rz width [rf zf rb zb]
NW = 2 * BL               # 32   per-step n width [nf nb]
# block index for (dir d, gate g): rz blocks 0..3, n blocks 0..1
_BLKRZ = {(0, 0): 0, (0, 1): 1, (1, 0): 2, (1, 1): 3}


def _sub(base, off, dims):
    """Manual AP: base is a [128, X] AP; append free dims after partition."""
    return bass.AP(tensor=base.tensor, offset=base.offset + off,
                   ap=[base.ap[0]] + dims)


def _v(t, dims, off=0):
    return bass.AP(tensor=t.tensor, offset=t.offset + off,
                   ap=[t.ap[0]] + dims)


def _split_waits(nc, cap=1):
    """Hoist excess sync waits onto standalone event-semaphore ops.

    The walrus build on this stack accepts only `cap` sync-wait commands
    per ISA instruction; Tile can attach several. Event-semaphore ops on
    the same engine execute in queue order, so hoisting preserves
    semantics.
    """
    n = 0
    for fn in nc.m.functions:
        for bb in fn.blocks:
            out = []
            for ins in bb.instructions:
                si = ins.sync_info
                if si is not None and len(si.on_wait) > cap:
                    waits = list(si.on_wait)
                    keep = waits[len(waits) - cap:] if cap else []
                    for w in waits[:len(waits) - cap] if cap else waits:
                        n += 1
                        out.append(mybir.InstEventSemaphore(
                            name=f"wsplit-{n}", engine=ins.engine,
                            ins=[], outs=[],
                            sync_info=mybir.SyncInfo(on_wait=[w],
                                                     on_update=[])))
                    ins.sync_info = mybir.SyncInfo(
                        on_wait=keep, on_update=list(si.on_update))
                out.append(ins)
            bb.instructions = out
    return n


def _build(zero_bhn: bool, debug: bool = False):
    nc = bass.Bass()
    if debug:
        dbg_hs_d = nc.declare_dram_parameter("dbg_hs", [128, 2 * (S + 1) * BL],
                                             F32, True)
        dbg_uh_d = nc.declare_dram_parameter("dbg_uh", [128, NGRP * 160], F32,
                                             True)
        dbg_bl_d = nc.declare_dram_parameter("dbg_bl", [128, NGRP * NUM_CAP],
                                             F32, True)
        dbg_o_d = nc.declare_dram_parameter("dbg_o", [BL, 160], F32, True)

    xidx_d = nc.declare_dram_parameter("xidx", [128, NGRP], I32, False)
    emb_d = nc.declare_dram_parameter("emb", [VOCAB, D_W], F32, False)
    wih_d = nc.declare_dram_parameter("wih", [2, D_W, G3], BF16, False)
    whh_d = nc.declare_dram_parameter("whh", [2, H, G3], BF16, False)
    biasx_d = nc.declare_dram_parameter("biasx", [128, 6], F32, False)
    bhn_d = nc.declare_dram_parameter("bhn", [128, 2], F32, False)
    wcap_d = nc.declare_dram_parameter("wcap", [2, H, 160], BF16, False)
    wlin_d = nc.declare_dram_parameter("wlin", [160, 2], F32, False)
    blin_d = nc.declare_dram_parameter("blin", [2, 1], F32, False)
    selB_d = nc.declare_dram_parameter("selB", [128, BL], F32, False)
    selT_d = nc.declare_dram_parameter("selT", [BL, 128], F32, False)
    ident_d = nc.declare_dram_parameter("ident", [128, 128], F32, False)
    out_d = nc.declare_dram_parameter("out", [BL, 2], F32, True)

    with tile.TileContext(nc) as tc, ExitStack() as ctx:
        const = ctx.enter_context(tc.tile_pool(name="const", bufs=1))
        bigxp = ctx.enter_context(tc.tile_pool(name="bigxp", bufs=1))
        bighs = ctx.enter_context(tc.tile_pool(name="bighs", bufs=1))
        work = ctx.enter_context(tc.tile_pool(name="work", bufs=3))

        # ---- constants to SBUF ----
        xidx = const.tile([128, NGRP], I32)
        nc.sync.dma_start(out=xidx[:], in_=xidx_d[:, :])
        whh = const.tile([128, 2, G3], BF16)
        for d in range(2):
            nc.sync.dma_start(out=whh[:, d, :], in_=whh_d[d, :, :])
        biasx = const.tile([128, 6], F32)
        nc.sync.dma_start(out=biasx[:], in_=biasx_d[:, :])
        bhn = const.tile([128, 2], F32)
        nc.sync.dma_start(out=bhn[:], in_=bhn_d[:, :])
        wcap = const.tile([128, 2, 160], BF16)
        for k in range(2):
            nc.sync.dma_start(out=wcap[:, k, :], in_=wcap_d[k, :, :])
        wlin = const.tile([128, 2, 2], F32)        # chunk0 [:128], chunk1 [:32]
        nc.sync.dma_start(out=wlin[:, 0, :], in_=wlin_d[0:128, :])
        nc.sync.dma_start(out=wlin[:32, 1, :], in_=wlin_d[128:160, :])
        blin = const.tile([2, 1], F32)
        nc.sync.dma_start(out=blin[:], in_=blin_d[:, :])
        selB = const.tile([128, BL], F32)
        nc.sync.dma_start(out=selB[:], in_=selB_d[:, :])
        selT = const.tile([BL, 128], F32)
        nc.sync.dma_start(out=selT[:], in_=selT_d[:, :])
        ident = const.tile([128, 128], F32)
        nc.sync.dma_start(out=ident[:], in_=ident_d[:, :])
        identb = const.tile([128, 128], BF16)
        nc.scalar.copy(identb[:], ident[:])
        epst = const.tile([128, 1], F32)
        nc.vector.memset(epst[:], EPS)

        xprz = bigxp.tile([128, S * RZW], BF16)     # 32.8 KB/part
        xpn = bigxp.tile([128, S * NW], F32)        # 32.8 KB/part
        HB0 = (S + 1) * BL
        hs = bighs.tile([128, 2 * (S + 1) * BL], F32)    # 32.9 KB/part
        hbf = bighs.tile([128, 2 * (S + 1) * BL], BF16)  # 16.4 KB/part

        # ---- phases B+C: gather + cast + transpose + x_proj, 2 half passes --
        HTOK = NTOK // 2
        with tc.tile_pool(name="bc", bufs=1) as bc, \
             tc.tile_pool(name="gat", bufs=4) as gat, \
             tc.tile_pool(name="ps_bc", bufs=1, space="PSUM") as ps_bc:
            wih = bc.tile([128, 2, 3, G3], BF16)   # [kpart, dir, kchunk, gcol]
            for d in range(2):
                for k, (k0, kn) in enumerate(KCH):
                    nc.sync.dma_start(out=wih[:kn, d, k, :],
                                      in_=wih_d[d, k0:k0 + kn, :])
            for half in range(2):
                eT = [bc.tile([128, HTOK], BF16, name=f"eT{k}", tag=f"eT{k}")
                      for k in range(3)]
                for i in range(NGRP // 2):
                    ig = half * (NGRP // 2) + i
                    g = gat.tile([128, D_W], F32, name="g", tag="g")
                    nc.gpsimd.indirect_dma_start(
                        out=g[:], out_offset=None,
                        in_=emb_d[:, :],
                        in_offset=IndirectOffsetOnAxis(ap=xidx[:, ig:ig + 1],
                                                       axis=0))
                    gb = gat.tile([128, D_W], BF16, name="gb", tag="gb")
                    nc.gpsimd.tensor_copy(gb[:], g[:])
                    for k, (k0, kn) in enumerate(KCH):
                        pt = ps_bc.tile([128, 128], F32, tag="ptr", bufs=2)
                        nc.tensor.matmul(pt[:kn, :], lhsT=gb[:, k0:k0 + kn],
                                         rhs=identb[:], start=True, stop=True)
                        if (i + k) % 2 == 0:
                            nc.vector.tensor_copy(
                                eT[k][:kn, i * 128:(i + 1) * 128], pt[:kn, :])
                        else:
                            nc.scalar.copy(
                                eT[k][:kn, i * 128:(i + 1) * 128], pt[:kn, :])
                for d in range(2):
                    for gt in range(3):
                        for ch in range(NCH // 2):
                            px = ps_bc.tile([128, 512], F32, tag="px", bufs=3)
                            for k, (k0, kn) in enumerate(KCH):
                                nc.tensor.matmul(
                                    px[:, :],
                                    lhsT=wih[:kn, d, k, gt * H:(gt + 1) * H],
                                    rhs=eT[k][:kn, ch * 512:(ch + 1) * 512],
                                    start=(k == 0), stop=(k == 2))
                            gch = half * (NCH // 2) + ch
                            src = _v(px, [[BL, 32], [1, BL]])
                            if gt < 2:
                                blk = _BLKRZ[(d, gt)]
                                dst = _sub(xprz[:], gch * 32 * RZW + blk * BL,
                                           [[RZW, 32], [1, BL]])
                                bcol = blk
                            else:
                                dst = _sub(xpn[:], gch * 32 * NW + d * BL,
                                           [[NW, 32], [1, BL]])
                                bcol = 4 + d
                            if (d * 3 + gt + ch) % 2 == 0:
                                nc.vector.tensor_scalar_add(
                                    dst, src, biasx[:, bcol:bcol + 1])
                            else:
                                nc.scalar.activation(
                                    dst, src, AF.Identity,
                                    bias=biasx[:, bcol:bcol + 1])

        # ---- phase D: scan ----
        nc.vector.memset(_sub(hs[:], 0, [[1, BL]]), 0.0)               # h_f(-1)
        nc.vector.memset(_sub(hs[:], HB0 + S * BL, [[1, BL]]), 0.0)    # h_b(S)
        nc.gpsimd.memset(_sub(hbf[:], 0, [[1, BL]]), 0.0)
        nc.gpsimd.memset(_sub(hbf[:], HB0 + S * BL, [[1, BL]]), 0.0)
        with tc.tile_pool(name="ps_scan", bufs=1, space="PSUM") as ps_sc:
            for t in range(S):
                prz = ps_sc.tile([128, RZW], F32, tag="prz", bufs=3)
                pn = ps_sc.tile([128, NW], F32, tag="pn", bufs=3)
                hfb = _sub(hbf[:], t * BL, [[1, BL]])
                hbb = _sub(hbf[:], HB0 + (S - t) * BL, [[1, BL]])
                # xp_rz preload via identity matmul (f at t, b at S-1-t)
                xrz = _sub(xprz[:], t * RZW,
                           [[(S - 1 - 2 * t) * RZW + 2 * BL, 2], [1, 2 * BL]])
                prev = nc.tensor.matmul(prz[:], lhsT=identb[:], rhs=xrz,
                                        start=True, stop=False)
                for j, (d, hcur) in enumerate(((0, hfb), (0, hfb),
                                               (1, hbb), (1, hbb))):
                    gt = j % 2
                    cb = _BLKRZ[(d, gt)]
                    gm = nc.tensor.matmul(
                        prz[:, cb * BL:(cb + 1) * BL],
                        lhsT=whh[:, d, gt * H:(gt + 1) * H],
                        rhs=hcur, start=False, stop=(j == 3))
                    add_dep_helper(gm.ins, prev.ins, sync=False,
                                   reason="psum accum order")
                    prev = gm
                for d, hcur in ((0, hfb), (1, hbb)):
                    nc.tensor.matmul(pn[:, d * BL:(d + 1) * BL],
                                     lhsT=whh[:, d, 2 * H:3 * H],
                                     rhs=hcur, start=True, stop=True)

                rw = work.tile([128, RZW], F32, tag="rw")
                nc.scalar.activation(rw[:], prz[:], AF.Sigmoid)
                # r blocks at 0,2 ; w blocks at 1,3 (w = 1-z, z pre-negated)
                r_v = _v(rw, [[2 * BL, 2], [1, BL]])
                w_v = _v(rw, [[2 * BL, 2], [1, BL]], off=BL)

                tn = work.tile([128, NW], F32, tag="tn")
                if zero_bhn:
                    nc.vector.tensor_tensor(_v(tn, [[BL, 2], [1, BL]]),
                                            _v(pn, [[BL, 2], [1, BL]]),
                                            r_v, op=OP.mult)
                else:
                    for d in range(2):
                        nc.vector.scalar_tensor_tensor(
                            _v(tn, [[1, BL]], off=d * BL),
                            _v(pn, [[1, BL]], off=d * BL),
                            bhn[:, d:d + 1],
                            _v(rw, [[1, BL]], off=2 * d * BL),
                            op0=OP.add, op1=OP.mult)
                t2 = work.tile([128, NW], F32, tag="t2")
                xn = _sub(xpn[:], t * NW,
                          [[(S - 1 - 2 * t) * NW + BL, 2], [1, BL]])
                nc.vector.tensor_add(_v(t2, [[BL, 2], [1, BL]]),
                                     _v(tn, [[BL, 2], [1, BL]]), xn)
                n_t = work.tile([128, NW], F32, tag="n_t")
                nc.scalar.activation(n_t[:], t2[:], AF.Tanh)

                dlt = HB0 + (S - 2 * t) * BL
                hprev = _sub(hs[:], t * BL, [[dlt, 2], [1, BL]])
                dltw = HB0 + (S - 2 * t - 2) * BL
                hnew = _sub(hs[:], (t + 1) * BL, [[dltw, 2], [1, BL]])
                hnewb = _sub(hbf[:], (t + 1) * BL, [[dltw, 2], [1, BL]])
                n_v = _v(n_t, [[BL, 2], [1, BL]])
                # h' = (h - w*h) + w*n : A and C computable right after the
                # sigmoid (off the tanh critical path); only B and the final
                # add trail the tanh.
                a_t = work.tile([128, NW], F32, tag="a_t")
                a_v = _v(a_t, [[BL, 2], [1, BL]])
                nc.gpsimd.tensor_tensor(a_v, w_v, hprev, op=OP.mult)
                c_w = work.tile([128, NW], F32, tag="c_w")
                c_v = _v(c_w, [[BL, 2], [1, BL]])
                nc.gpsimd.tensor_tensor(c_v, hprev, a_v, op=OP.subtract)
                b_t = work.tile([128, NW], F32, tag="b_t")
                b_v = _v(b_t, [[BL, 2], [1, BL]])
                nc.vector.tensor_tensor(b_v, w_v, n_v, op=OP.mult)
                nc.vector.tensor_tensor(hnew, c_v, b_v, op=OP.add)
                nc.gpsimd.tensor_tensor(hnewb, c_v, b_v, op=OP.add)

        if debug:
            nc.sync.dma_start(out=dbg_hs_d[:, :], in_=hs[:])

        # ---- phases E/F/G ----
        with tc.tile_pool(name="ef", bufs=1) as ef, \
             tc.tile_pool(name="rp", bufs=1) as rp, \
             tc.tile_pool(name="ps_ef", bufs=1, space="PSUM") as ps_ef:
            # capsule u_hat [sb, 160]
            uh = ef.tile([128, NGRP * 160], F32)
            for c in range(NGRP):
                pu = ps_ef.tile([128, 160], F32, tag="pu", bufs=2)
                lhs_f = _sub(hbf[:], (1 + 8 * c) * BL, [[1, 128]])
                lhs_b = _sub(hbf[:], HB0 + 8 * c * BL, [[1, 128]])
                nc.tensor.matmul(pu[:], lhsT=lhs_f, rhs=wcap[:, 0, :],
                                 start=True, stop=False)
                nc.tensor.matmul(pu[:], lhsT=lhs_b, rhs=wcap[:, 1, :],
                                 start=False, stop=True)
                if c % 2 == 0:
                    nc.vector.tensor_copy(uh[:, c * 160:(c + 1) * 160], pu[:])
                else:
                    nc.scalar.copy(uh[:, c * 160:(c + 1) * 160], pu[:])

            if debug:
                nc.sync.dma_start(out=dbg_uh_d[:, :], in_=uh[:])

            # routing
            c_t = rp.tile([128, NGRP * NUM_CAP], F32, tag="c")   # [p, ch, cap]
            nc.vector.memset(c_t[:], 1.0 / NUM_CAP)
            bl_t = rp.tile([128, NGRP * NUM_CAP], F32, tag="bl")
            nc.gpsimd.memset(bl_t[:], 0.0)
            outputs = rp.tile([BL, 160], F32, tag="outs")
            tmp = rp.tile([128, NGRP * 160], F32, tag="tmp")

            for it in range(ROUTINGS):
                if it > 0:
                    # softmax over cap (free groups of 10)
                    mx = rp.tile([128, NGRP], F32, tag="mx", bufs=2)
                    nc.vector.tensor_reduce(
                        mx[:], _v(bl_t, [[NUM_CAP, NGRP], [1, NUM_CAP]]),
                        axis=AX.X, op=OP.max)
                    sb_t = rp.tile([128, NGRP * NUM_CAP], F32, tag="sb",
                                   bufs=2)
                    nc.vector.tensor_tensor(
                        _v(sb_t, [[NUM_CAP, NGRP], [1, NUM_CAP]]),
                        _v(bl_t, [[NUM_CAP, NGRP], [1, NUM_CAP]]),
                        _v(mx, [[1, NGRP], [0, NUM_CAP]]), op=OP.subtract)
                    nc.scalar.activation(sb_t[:], sb_t[:], AF.Exp)
                    sm = rp.tile([128, NGRP], F32, tag="sm", bufs=2)
                    nc.vector.tensor_reduce(
                        sm[:], _v(sb_t, [[NUM_CAP, NGRP], [1, NUM_CAP]]),
                        axis=AX.X, op=OP.add)
                    rc = rp.tile([128, NGRP], F32, tag="rc", bufs=2)
                    nc.vector.reciprocal(rc[:], sm[:])
                    nc.vector.tensor_tensor(
                        _v(c_t, [[NUM_CAP, NGRP], [1, NUM_CAP]]),
                        _v(sb_t, [[NUM_CAP, NGRP], [1, NUM_CAP]]),
                        _v(rc, [[1, NGRP], [0, NUM_CAP]]), op=OP.mult)

                # tmp = u_hat * c (c broadcast over dc), sum over s via matmul
                po = ps_ef.tile([BL, 160], F32, tag="po", bufs=2)
                for half in range(2):
                    lo = half * (NGRP // 2)
                    eng = nc.vector if half == 0 else nc.gpsimd
                    eng.tensor_tensor(
                        _sub(tmp[:], lo * 160,
                             [[160, NGRP // 2], [DIM_CAP, NUM_CAP],
                              [1, DIM_CAP]]),
                        _sub(uh[:], lo * 160,
                             [[160, NGRP // 2], [DIM_CAP, NUM_CAP],
                              [1, DIM_CAP]]),
                        _sub(c_t[:], lo * NUM_CAP,
                             [[NUM_CAP, NGRP // 2], [1, NUM_CAP],
                              [0, DIM_CAP]]),
                        op=OP.mult)
                for j in range(NGRP):
                    nc.tensor.matmul(po[:], lhsT=selB[:],
                                     rhs=tmp[:, j * 160:(j + 1) * 160],
                                     start=(j == 0), stop=(j == NGRP - 1))
                # squash
                sq = rp.tile([BL, 160], F32, tag="sq", bufs=2)
                nc.scalar.square(sq[:], po[:])
                ssum = rp.tile([BL, NUM_CAP], F32, tag="ssum", bufs=2)
                nc.vector.tensor_reduce(
                    ssum[:], _v(sq, [[DIM_CAP, NUM_CAP], [1, DIM_CAP]]),
                    axis=AX.X, op=OP.add)
                srt = rp.tile([BL, NUM_CAP], F32, tag="srt", bufs=2)
                nc.scalar.activation(srt[:], ssum[:], AF.Sqrt,
                                     bias=epst[:BL, 0:1])
                rs = rp.tile([BL, NUM_CAP], F32, tag="rs", bufs=2)
                nc.vector.reciprocal(rs[:], srt[:])
                nc.vector.tensor_tensor(
                    _v(outputs, [[DIM_CAP, NUM_CAP], [1, DIM_CAP]]),
                    _v(po, [[DIM_CAP, NUM_CAP], [1, DIM_CAP]]),
                    _v(rs, [[1, NUM_CAP], [0, DIM_CAP]]), op=OP.mult)

                if it < ROUTINGS - 1:
                    # broadcast outputs to all 128 partitions via selT matmul
                    pob = ps_ef.tile([128, 160], F32, tag="pob", bufs=1)
                    nc.tensor.matmul(pob[:], lhsT=selT[:], rhs=outputs[:],
                                     start=True, stop=True)
                    ob = rp.tile([128, 160], F32, tag="ob", bufs=2)
                    nc.scalar.copy(ob[:], pob[:])
                    # tmp = u_hat * ob (ob broadcast over chunks)
                    for half in range(2):
                        lo = half * (NGRP // 2)
                        eng = nc.vector if half == 0 else nc.gpsimd
                        eng.tensor_tensor(
                            _sub(tmp[:], lo * 160,
                                 [[160, NGRP // 2], [1, 160]]),
                            _sub(uh[:], lo * 160,
                                 [[160, NGRP // 2], [1, 160]]),
                            _v(ob, [[0, NGRP // 2], [1, 160]]),
                            op=OP.mult)
                    # du = sum over dc (innermost 16) -> [128, NGRP*NUM_CAP]
                    du = rp.tile([128, NGRP * NUM_CAP], F32, tag="du", bufs=2)
                    nc.vector.tensor_reduce(
                        _v(du, [[NUM_CAP, NGRP], [1, NUM_CAP]]),
                        _v(tmp, [[160, NGRP], [DIM_CAP, NUM_CAP],
                                 [1, DIM_CAP]]),
                        axis=AX.X, op=OP.add)
                    nc.vector.tensor_add(bl_t[:], bl_t[:], du[:])

            if debug:
                nc.sync.dma_start(out=dbg_bl_d[:, :], in_=bl_t[:])
                nc.sync.dma_start(out=dbg_o_d[:, :], in_=outputs[:])

            # final linear
            pt1 = ps_ef.tile([128, BL], F32, tag="pt1", bufs=1)
            nc.tensor.matmul(pt1[:, :], lhsT=outputs[:, 0:128],
                             rhs=ident[:BL, :BL], start=True, stop=True)
            pt2 = ps_ef.tile([32, BL], F32, tag="pt2", bufs=1)
            nc.tensor.matmul(pt2[:, :], lhsT=outputs[:, 128:160],
                             rhs=ident[:BL, :BL], start=True, stop=True)
            capsT = rp.tile([128, 2 * BL], F32, tag="capsT")
            nc.vector.tensor_copy(capsT[:, 0:BL], pt1[:])
            nc.vector.tensor_copy(capsT[:32, BL:2 * BL], pt2[:])
            pf = ps_ef.tile([2, BL], F32, tag="pf", bufs=1)
            nc.tensor.matmul(pf[:], lhsT=wlin[:, 0, :], rhs=capsT[:, 0:BL],
                             start=True, stop=False)
            nc.tensor.matmul(pf[:], lhsT=wlin[:32, 1, :],
                             rhs=capsT[:32, BL:2 * BL],
                             start=False, stop=True)
            outT = rp.tile([2, BL], F32, tag="outT")
            nc.scalar.activation(outT[:], pf[:], AF.Identity,
                                 bias=blin[:, 0:1])
            dst = bass.AP(tensor=out_d, offset=0, ap=[[1, 2], [2, BL]])
            nc.sync.dma_start(out=dst, in_=outT[:])

    return nc


_CACHE = {}


def _get_nc(zero_bhn):
    if zero_bhn not in _CACHE:
        nc = _build(zero_bhn)
        _split_waits(nc)   # HW-path legalization (CoreSim path builds its own)
        _CACHE[zero_bhn] = nc
    return _CACHE[zero_bhn]


def _host_inputs(x, emb, w_ih_f, w_hh_f, b_ih_f, b_hh_f,
                 w_ih_b, w_hh_b, b_ih_b, b_hh_b, W_cap, W_lin, b_lin):
    """Build the per-core input maps (everything but xidx is shared)."""
    import ml_dtypes
    f32 = np.float32
    bf16 = np.float16
    neg = np.ones((G3,), f32)
    neg[H:2 * H] = -1.0        # negate z gate (sigmoid -> 1-z)

    wih = np.stack([(w_ih_f.T * neg).astype(bf16), (w_ih_b.T * neg).astype(bf16)])
    whh = np.stack([(w_hh_f.T * neg).astype(bf16), (w_hh_b.T * neg).astype(bf16)])

    biasx = np.zeros((128, 6), f32)
    for d, (bi, bh) in enumerate([(b_ih_f, b_hh_f), (b_ih_b, b_hh_b)]):
        biasx[:, _BLKRZ[(d, 0)]] = (bi[0:H] + bh[0:H])
        biasx[:, _BLKRZ[(d, 1)]] = -(bi[H:2 * H] + bh[H:2 * H])
        biasx[:, 4 + d] = bi[2 * H:3 * H]
    bhn = np.zeros((128, 2), f32)
    bhn[:, 0] = b_hh_f[2 * H:3 * H]
    bhn[:, 1] = b_hh_b[2 * H:3 * H]
    zero_bhn = bool(np.all(bhn == 0.0))

    wcap = np.stack([W_cap[0:H, :].astype(bf16), W_cap[H:2 * H, :].astype(bf16)])
    selB = (np.arange(128)[:, None] % BL == np.arange(BL)[None, :]).astype(f32)
    selT = selB.T.copy()
    ident = np.eye(128, dtype=f32)

    shared = dict(emb=np.ascontiguousarray(emb, f32), wih=wih, whh=whh,
                  biasx=biasx, bhn=bhn, wcap=wcap,
                  wlin=np.ascontiguousarray(W_lin, f32),
                  blin=np.ascontiguousarray(b_lin, f32).reshape(2, 1),
                  selB=selB, selT=selT, ident=ident)

    in_maps = []
    for c in range(NCORES):
        xl = np.asarray(x[c * BL:(c + 1) * BL, :])          # [BL, S]
        tok = xl.T.reshape(-1).astype(np.int32)             # s-major [NTOK]
        xidx = np.ascontiguousarray(tok.reshape(NGRP, 128).T)  # [128, NGRP]
        in_maps.append(dict(shared, xidx=xidx))
    return in_maps, zero_bhn


def kernel(**inputs):
    in_maps, zero_bhn = _host_inputs(**{k: np.asarray(v) for k, v in
                                        inputs.items()})
    nc = _get_nc(zero_bhn)
    res = run_bass_kernel_spmd(nc, in_maps, list(range(NCORES)))
    return np.concatenate([res.results[c]["out"] for c in range(NCORES)],
                          axis=0)


def _install_ntff_hook():
    """Shim the missing antenv.axon_hooks so trace=True works under axon."""
    import sys, types
    if "antenv.axon_hooks" in sys.modules:
        return
    mod = types.ModuleType("antenv.axon_hooks")
    _h = [None]
    mod.set_axon_ntff_profile_hook = lambda h: _h.__setitem__(0, h)
    mod.get_axon_ntff_profile_hook = lambda: _h[0]
    sys.modules["antenv.axon_hooks"] = mod
    import antenv
    antenv.axon_hooks = mod
    from trn_agent_boot.trn_boot import _ntff_profile_via_ctypes
    mod.set_axon_ntff_profile_hook(
        _ntff_profile_via_ctypes("/opt/axon/libaxon_pjrt.so"))


def kernel_profiled(**inputs):
    """Same as kernel() but with NTFF tracing; returns (out, result_obj)."""
    _install_ntff_hook()
    in_maps, zero_bhn = _host_inputs(**{k: np.asarray(v) for k, v in
                                        inputs.items()})
    nc = _get_nc(zero_bhn)
    res = run_bass_kernel_spmd(nc, in_maps, list(range(NCORES)), trace=True)
    out = np.concatenate([res.results[c]["out"] for c in range(NCORES)],
                         axis=0)
    return out, res


# revision 32
# speedup vs baseline: 1.8353x; 1.0332x over previous
"""Trainium2 Bass kernel for nn_CapRNNModelHelper (bi-GRU + capsule routing).

Sharding: data-parallel over batch across 8 cores (16 batch rows per core).
Everything else (embedding table, GRU weights, capsule weights) replicated.

Per-core pipeline (v2, bf16 matmul operands, f32 accumulation):
  1. indirect-DMA gather of embedding rows (token order s-major), cast bf16
  2. PE-transpose (plain matmul vs identity) -> e.T  [300, ntok] bf16
  3. x_proj matmuls (bf16) -> xp_rz (bf16) + xp_n (f32), biases folded,
     z blocks negated so sigmoid gives w = 1-z directly
  4. 256-step fused bidirectional GRU scan: per step the xp_rz slice is
     PSUM-accumulated via an identity matmul, gates matmul on top (bf16
     weights, bf16 h mirror), sigmoid/tanh on ACT, update on DVE+GpSimd
  5. capsule matmul (bf16) -> u_hat [sb, 160] f32
  6. 5-iter dynamic routing (selector matmuls for sequence reductions)
  7. final linear -> out [16, 2]
"""

import numpy as np
from contextlib import ExitStack

import concourse.bass as bass
import concourse.tile as tile
from concourse import mybir
from concourse.bass import IndirectOffsetOnAxis
from concourse.bass_utils import run_bass_kernel_spmd
from concourse.tile_rust import add_dep_helper

F32 = mybir.dt.float32
BF16 = mybir.dt.float16
I32 = mybir.dt.int32
AF = mybir.ActivationFunctionType
OP = mybir.AluOpType
AX = mybir.AxisListType

VOCAB, D_W, H, S, B = 50000, 300, 128, 256, 128
NUM_CAP, DIM_CAP, ROUTINGS, EPS = 10, 16, 5, 1e-7
NCORES = 8
BL = B // NCORES          # 16 batch rows per core
NTOK = S * BL             # 4096 tokens per core
NGRP = NTOK // 128        # 32 gather groups of 128 tokens
NCH = NTOK // 512         # 8 x_proj chunks of 512 tokens
KCH = [(0, 128), (128, 128), (256, 44)]   # D_W split
G3 = 3 * H                # 384

RZW = 4 * BL              # 64   per-step rz width [rf zf rb zb]
NW = 2 * BL               # 32   per-step n width [nf nb]
# block index for (dir d, gate g): rz blocks 0..3, n blocks 0..1
_BLKRZ = {(0, 0): 0, (0, 1): 1, (1, 0): 2, (1, 1): 3}


def _sub(base, off, dims):
    """Manual AP: base is a [128, X] AP; append free dims after partition."""
    return bass.AP(tensor=base.tensor, offset=base.offset + off,
                   ap=[base.ap[0]] + dims)


def _v(t, dims, off=0):
    return bass.AP(tensor=t.tensor, offset=t.offset + off,
                   ap=[t.ap[0]] + dims)


def _split_waits(nc, cap=1):
    """Hoist excess sync waits onto standalone event-semaphore ops.

    The walrus build on this stack accepts only `cap` sync-wait commands
    per ISA instruction; Tile can attach several. Event-semaphore ops on
    the same engine execute in queue order, so hoisting preserves
    semantics.
    """
    n = 0
    for fn in nc.m.functions:
        for bb in fn.blocks:
            out = []
            for ins in bb.instructions:
                si = ins.sync_info
                if si is not None and len(si.on_wait) > cap:
                    waits = list(si.on_wait)
                    keep = waits[len(waits) - cap:] if cap else []
                    for w in waits[:len(waits) - cap] if cap else waits:
                        n += 1
                        out.append(mybir.InstEventSemaphore(
                            name=f"wsplit-{n}", engine=ins.engine,
                            ins=[], outs=[],
                            sync_info=mybir.SyncInfo(on_wait=[w],
                                                     on_update=[])))
                    ins.sync_info = mybir.SyncInfo(
                        on_wait=keep, on_update=list(si.on_update))
                out.append(ins)
            bb.instructions = out
    return n


def _build(zero_bhn: bool, debug: bool = False):
    nc = bass.Bass()
    if debug:
        dbg_hs_d = nc.declare_dram_parameter("dbg_hs", [128, 2 * (S + 1) * BL],
                                             F32, True)
        dbg_uh_d = nc.declare_dram_parameter("dbg_uh", [128, NGRP * 160], F32,
                                             True)
        dbg_bl_d = nc.declare_dram_parameter("dbg_bl", [128, NGRP * NUM_CAP],
                                             F32, True)
        dbg_o_d = nc.declare_dram_parameter("dbg_o", [BL, 160], F32, True)

    xidx_d = nc.declare_dram_parameter("xidx", [128, NGRP], I32, False)
    emb_d = nc.declare_dram_parameter("emb", [VOCAB, D_W], F32, False)
    wih_d = nc.declare_dram_parameter("wih", [2, D_W, G3], BF16, False)
    whh_d = nc.declare_dram_parameter("whh", [2, H, G3], BF16, False)
    biasx_d = nc.declare_dram_parameter("biasx", [128, 6], F32, False)
    bhn_d = nc.declare_dram_parameter("bhn", [128, 2], F32, False)
    wcap_d = nc.declare_dram_parameter("wcap", [2, H, 160], BF16, False)
    wlin_d = nc.declare_dram_parameter("wlin", [160, 2], F32, False)
    blin_d = nc.declare_dram_parameter("blin", [2, 1], F32, False)
    selB_d = nc.declare_dram_parameter("selB", [128, BL], F32, False)
    selT_d = nc.declare_dram_parameter("selT", [BL, 128], F32, False)
    ident_d = nc.declare_dram_parameter("ident", [128, 128], F32, False)
    out_d = nc.declare_dram_parameter("out", [BL, 2], F32, True)

    with tile.TileContext(nc) as tc, ExitStack() as ctx:
        const = ctx.enter_context(tc.tile_pool(name="const", bufs=1))
        bigxp = ctx.enter_context(tc.tile_pool(name="bigxp", bufs=1))
        bighs = ctx.enter_context(tc.tile_pool(name="bighs", bufs=1))
        work = ctx.enter_context(tc.tile_pool(name="work", bufs=3))

        # ---- constants to SBUF ----
        xidx = const.tile([128, NGRP], I32)
        nc.sync.dma_start(out=xidx[:], in_=xidx_d[:, :])
        whh = const.tile([128, 2, G3], BF16)
        for d in range(2):
            nc.sync.dma_start(out=whh[:, d, :], in_=whh_d[d, :, :])
        biasx = const.tile([128, 6], F32)
        nc.sync.dma_start(out=biasx[:], in_=biasx_d[:, :])
        bhn = const.tile([128, 2], F32)
        nc.sync.dma_start(out=bhn[:], in_=bhn_d[:, :])
        wcap = const.tile([128, 2, 160], BF16)
        for k in range(2):
            nc.sync.dma_start(out=wcap[:, k, :], in_=wcap_d[k, :, :])
        wlin = const.tile([128, 2, 2], F32)        # chunk0 [:128], chunk1 [:32]
        nc.sync.dma_start(out=wlin[:, 0, :], in_=wlin_d[0:128, :])
        nc.sync.dma_start(out=wlin[:32, 1, :], in_=wlin_d[128:160, :])
        blin = const.tile([2, 1], F32)
        nc.sync.dma_start(out=blin[:], in_=blin_d[:, :])
        selB = const.tile([128, BL], F32)
        nc.sync.dma_start(out=selB[:], in_=selB_d[:, :])
        selT = const.tile([BL, 128], F32)
        nc.sync.dma_start(out=selT[:], in_=selT_d[:, :])
        ident = const.tile([128, 128], F32)
        nc.sync.dma_start(out=ident[:], in_=ident_d[:, :])
        identb = const.tile([128, 128], BF16)
        nc.scalar.copy(identb[:], ident[:])
        epst = const.tile([128, 1], F32)
        nc.vector.memset(epst[:], EPS)

        xprz = bigxp.tile([128, S * RZW], BF16)     # 32.8 KB/part
        xpn = bigxp.tile([128, S * NW], F32)        # 32.8 KB/part
        HB0 = (S + 1) * BL
        hs = bighs.tile([128, 2 * (S + 1) * BL], F32)    # 32.9 KB/part
        hbf = bighs.tile([128, 2 * (S + 1) * BL], BF16)  # 16.4 KB/part

        # ---- phases B+C: gather + cast + transpose + x_proj, 2 half passes --
        HTOK = NTOK // 2
        with tc.tile_pool(name="bc", bufs=1) as bc, \
             tc.tile_pool(name="gat", bufs=4) as gat, \
             tc.tile_pool(name="ps_bc", bufs=1, space="PSUM") as ps_bc:
            wih = bc.tile([128, 2, 3, G3], BF16)   # [kpart, dir, kchunk, gcol]
            for d in range(2):
                for k, (k0, kn) in enumerate(KCH):
                    nc.sync.dma_start(out=wih[:kn, d, k, :],
                                      in_=wih_d[d, k0:k0 + kn, :])
            for half in range(2):
                eT = [bc.tile([128, HTOK], BF16, name=f"eT{k}", tag=f"eT{k}")
                      for k in range(3)]
                for i in range(NGRP // 2):
                    ig = half * (NGRP // 2) + i
                    g = gat.tile([128, D_W], F32, name="g", tag="g")
                    nc.gpsimd.indirect_dma_start(
                        out=g[:], out_offset=None,
                        in_=emb_d[:, :],
                        in_offset=IndirectOffsetOnAxis(ap=xidx[:, ig:ig + 1],
                                                       axis=0))
                    gb = gat.tile([128, D_W], BF16, name="gb", tag="gb")
                    nc.gpsimd.tensor_copy(gb[:], g[:])
                    for k, (k0, kn) in enumerate(KCH):
                        pt = ps_bc.tile([128, 128], F32, tag="ptr", bufs=2)
                        nc.tensor.matmul(pt[:kn, :], lhsT=gb[:, k0:k0 + kn],
                                         rhs=identb[:], start=True, stop=True)
                        if (i + k) % 2 == 0:
                            nc.vector.tensor_copy(
                                eT[k][:kn, i * 128:(i + 1) * 128], pt[:kn, :])
                        else:
                            nc.scalar.copy(
                                eT[k][:kn, i * 128:(i + 1) * 128], pt[:kn, :])
                for d in range(2):
                    for gt in range(3):
                        for ch in range(NCH // 2):
                            px = ps_bc.tile([128, 512], F32, tag="px", bufs=3)
                            for k, (k0, kn) in enumerate(KCH):
                                nc.tensor.matmul(
                                    px[:, :],
                                    lhsT=wih[:kn, d, k, gt * H:(gt + 1) * H],
                                    rhs=eT[k][:kn, ch * 512:(ch + 1) * 512],
                                    start=(k == 0), stop=(k == 2))
                            gch = half * (NCH // 2) + ch
                            src = _v(px, [[BL, 32], [1, BL]])
                            if gt < 2:
                                blk = _BLKRZ[(d, gt)]
                                dst = _sub(xprz[:], gch * 32 * RZW + blk * BL,
                                           [[RZW, 32], [1, BL]])
                                bcol = blk
                            else:
                                dst = _sub(xpn[:], gch * 32 * NW + d * BL,
                                           [[NW, 32], [1, BL]])
                                bcol = 4 + d
                            if (d * 3 + gt + ch) % 2 == 0:
                                nc.vector.tensor_scalar_add(
                                    dst, src, biasx[:, bcol:bcol + 1])
                            else:
                                nc.scalar.activation(
                                    dst, src, AF.Identity,
                                    bias=biasx[:, bcol:bcol + 1])

        # ---- phase D: scan ----
        nc.vector.memset(_sub(hs[:], 0, [[1, BL]]), 0.0)               # h_f(-1)
        nc.vector.memset(_sub(hs[:], HB0 + S * BL, [[1, BL]]), 0.0)    # h_b(S)
        nc.gpsimd.memset(_sub(hbf[:], 0, [[1, BL]]), 0.0)
        nc.gpsimd.memset(_sub(hbf[:], HB0 + S * BL, [[1, BL]]), 0.0)
        with tc.tile_pool(name="ps_scan", bufs=1, space="PSUM") as ps_sc:
            for t in range(S):
                prz = ps_sc.tile([128, RZW], F32, tag="prz", bufs=4)
                pn = ps_sc.tile([128, NW], F32, tag="pn", bufs=4)
                hfb = _sub(hbf[:], t * BL, [[1, BL]])
                hbb = _sub(hbf[:], HB0 + (S - t) * BL, [[1, BL]])
                # xp_rz preload via identity matmul (f at t, b at S-1-t)
                xrz = _sub(xprz[:], t * RZW,
                           [[(S - 1 - 2 * t) * RZW + 2 * BL, 2], [1, 2 * BL]])
                prev = nc.tensor.matmul(prz[:], lhsT=identb[:], rhs=xrz,
                                        start=True, stop=False)
                for j, (d, hcur) in enumerate(((0, hfb), (0, hfb),
                                               (1, hbb), (1, hbb))):
                    gt = j % 2
                    cb = _BLKRZ[(d, gt)]
                    gm = nc.tensor.matmul(
                        prz[:, cb * BL:(cb + 1) * BL],
                        lhsT=whh[:, d, gt * H:(gt + 1) * H],
                        rhs=hcur, start=False, stop=(j == 3))
                    add_dep_helper(gm.ins, prev.ins, sync=False,
                                   reason="psum accum order")
                    prev = gm
                for d, hcur in ((0, hfb), (1, hbb)):
                    nc.tensor.matmul(pn[:, d * BL:(d + 1) * BL],
                                     lhsT=whh[:, d, 2 * H:3 * H],
                                     rhs=hcur, start=True, stop=True)

                rw = work.tile([128, RZW], F32, tag="rw")
                nc.scalar.activation(rw[:], prz[:], AF.Sigmoid)
                # r blocks at 0,2 ; w blocks at 1,3 (w = 1-z, z pre-negated)
                r_v = _v(rw, [[2 * BL, 2], [1, BL]])
                w_v = _v(rw, [[2 * BL, 2], [1, BL]], off=BL)

                tn = work.tile([128, NW], F32, tag="tn")
                if zero_bhn:
                    nc.vector.tensor_tensor(_v(tn, [[BL, 2], [1, BL]]),
                                            _v(pn, [[BL, 2], [1, BL]]),
                                            r_v, op=OP.mult)
                else:
                    for d in range(2):
                        nc.vector.scalar_tensor_tensor(
                            _v(tn, [[1, BL]], off=d * BL),
                            _v(pn, [[1, BL]], off=d * BL),
                            bhn[:, d:d + 1],
                            _v(rw, [[1, BL]], off=2 * d * BL),
                            op0=OP.add, op1=OP.mult)
                t2 = work.tile([128, NW], F32, tag="t2")
                xn = _sub(xpn[:], t * NW,
                          [[(S - 1 - 2 * t) * NW + BL, 2], [1, BL]])
                nc.vector.tensor_add(_v(t2, [[BL, 2], [1, BL]]),
                                     _v(tn, [[BL, 2], [1, BL]]), xn)
                n_t = work.tile([128, NW], F32, tag="n_t")
                nc.scalar.activation(n_t[:], t2[:], AF.Tanh)

                dlt = HB0 + (S - 2 * t) * BL
                hprev = _sub(hs[:], t * BL, [[dlt, 2], [1, BL]])
                dltw = HB0 + (S - 2 * t - 2) * BL
                hnew = _sub(hs[:], (t + 1) * BL, [[dltw, 2], [1, BL]])
                hnewb = _sub(hbf[:], (t + 1) * BL, [[dltw, 2], [1, BL]])
                n_v = _v(n_t, [[BL, 2], [1, BL]])
                # h' = (h - w*h) + w*n : A and C computable right after the
                # sigmoid (off the tanh critical path); only B and the final
                # add trail the tanh.
                a_t = work.tile([128, NW], F32, tag="a_t")
                a_v = _v(a_t, [[BL, 2], [1, BL]])
                nc.gpsimd.tensor_tensor(a_v, w_v, hprev, op=OP.mult)
                c_w = work.tile([128, NW], F32, tag="c_w")
                c_v = _v(c_w, [[BL, 2], [1, BL]])
                nc.gpsimd.tensor_tensor(c_v, hprev, a_v, op=OP.subtract)
                b_t = work.tile([128, NW], F32, tag="b_t")
                b_v = _v(b_t, [[BL, 2], [1, BL]])
                nc.vector.tensor_tensor(b_v, w_v, n_v, op=OP.mult)
                nc.vector.tensor_tensor(hnewb, c_v, b_v, op=OP.add)
                nc.gpsimd.tensor_tensor(hnew, c_v, b_v, op=OP.add)

        if debug:
            nc.sync.dma_start(out=dbg_hs_d[:, :], in_=hs[:])

        # ---- phases E/F/G ----
        with tc.tile_pool(name="ef", bufs=1) as ef, \
             tc.tile_pool(name="rp", bufs=1) as rp, \
             tc.tile_pool(name="ps_ef", bufs=1, space="PSUM") as ps_ef:
            # capsule u_hat [sb, 160]
            uh = ef.tile([128, NGRP * 160], F32)
            for c in range(NGRP):
                pu = ps_ef.tile([128, 160], F32, tag="pu", bufs=2)
                lhs_f = _sub(hbf[:], (1 + 8 * c) * BL, [[1, 128]])
                lhs_b = _sub(hbf[:], HB0 + 8 * c * BL, [[1, 128]])
                nc.tensor.matmul(pu[:], lhsT=lhs_f, rhs=wcap[:, 0, :],
                                 start=True, stop=False)
                nc.tensor.matmul(pu[:], lhsT=lhs_b, rhs=wcap[:, 1, :],
                                 start=False, stop=True)
                if c % 2 == 0:
                    nc.vector.tensor_copy(uh[:, c * 160:(c + 1) * 160], pu[:])
                else:
                    nc.scalar.copy(uh[:, c * 160:(c + 1) * 160], pu[:])

            if debug:
                nc.sync.dma_start(out=dbg_uh_d[:, :], in_=uh[:])

            # routing
            c_t = rp.tile([128, NGRP * NUM_CAP], F32, tag="c")   # [p, ch, cap]
            nc.vector.memset(c_t[:], 1.0 / NUM_CAP)
            bl_t = rp.tile([128, NGRP * NUM_CAP], F32, tag="bl")
            nc.gpsimd.memset(bl_t[:], 0.0)
            outputs = rp.tile([BL, 160], F32, tag="outs")
            tmp = rp.tile([128, NGRP * 160], F32, tag="tmp")

            for it in range(ROUTINGS):
                if it > 0:
                    # softmax over cap (free groups of 10)
                    mx = rp.tile([128, NGRP], F32, tag="mx", bufs=2)
                    nc.vector.tensor_reduce(
                        mx[:], _v(bl_t, [[NUM_CAP, NGRP], [1, NUM_CAP]]),
                        axis=AX.X, op=OP.max)
                    sb_t = rp.tile([128, NGRP * NUM_CAP], F32, tag="sb",
                                   bufs=2)
                    nc.vector.tensor_tensor(
                        _v(sb_t, [[NUM_CAP, NGRP], [1, NUM_CAP]]),
                        _v(bl_t, [[NUM_CAP, NGRP], [1, NUM_CAP]]),
                        _v(mx, [[1, NGRP], [0, NUM_CAP]]), op=OP.subtract)
                    nc.scalar.activation(sb_t[:], sb_t[:], AF.Exp)
                    sm = rp.tile([128, NGRP], F32, tag="sm", bufs=2)
                    nc.vector.tensor_reduce(
                        sm[:], _v(sb_t, [[NUM_CAP, NGRP], [1, NUM_CAP]]),
                        axis=AX.X, op=OP.add)
                    rc = rp.tile([128, NGRP], F32, tag="rc", bufs=2)
                    nc.vector.reciprocal(rc[:], sm[:])
                    nc.vector.tensor_tensor(
                        _v(c_t, [[NUM_CAP, NGRP], [1, NUM_CAP]]),
                        _v(sb_t, [[NUM_CAP, NGRP], [1, NUM_CAP]]),
                        _v(rc, [[1, NGRP], [0, NUM_CAP]]), op=OP.mult)

                # tmp = u_hat * c (c broadcast over dc), sum over s via matmul
                po = ps_ef.tile([BL, 160], F32, tag="po", bufs=2)
                for half in range(2):
                    lo = half * (NGRP // 2)
                    eng = nc.vector if half == 0 else nc.gpsimd
                    eng.tensor_tensor(
                        _sub(tmp[:], lo * 160,
                             [[160, NGRP // 2], [DIM_CAP, NUM_CAP],
                              [1, DIM_CAP]]),
                        _sub(uh[:], lo * 160,
                             [[160, NGRP // 2], [DIM_CAP, NUM_CAP],
                              [1, DIM_CAP]]),
                        _sub(c_t[:], lo * NUM_CAP,
                             [[NUM_CAP, NGRP // 2], [1, NUM_CAP],
                              [0, DIM_CAP]]),
                        op=OP.mult)
                for j in range(NGRP):
                    nc.tensor.matmul(po[:], lhsT=selB[:],
                                     rhs=tmp[:, j * 160:(j + 1) * 160],
                                     start=(j == 0), stop=(j == NGRP - 1))
                # squash
                sq = rp.tile([BL, 160], F32, tag="sq", bufs=2)
                nc.scalar.square(sq[:], po[:])
                ssum = rp.tile([BL, NUM_CAP], F32, tag="ssum", bufs=2)
                nc.vector.tensor_reduce(
                    ssum[:], _v(sq, [[DIM_CAP, NUM_CAP], [1, DIM_CAP]]),
                    axis=AX.X, op=OP.add)
                srt = rp.tile([BL, NUM_CAP], F32, tag="srt", bufs=2)
                nc.scalar.activation(srt[:], ssum[:], AF.Sqrt,
                                     bias=epst[:BL, 0:1])
                rs = rp.tile([BL, NUM_CAP], F32, tag="rs", bufs=2)
                nc.vector.reciprocal(rs[:], srt[:])
                nc.vector.tensor_tensor(
                    _v(outputs, [[DIM_CAP, NUM_CAP], [1, DIM_CAP]]),
                    _v(po, [[DIM_CAP, NUM_CAP], [1, DIM_CAP]]),
                    _v(rs, [[1, NUM_CAP], [0, DIM_CAP]]), op=OP.mult)

                if it < ROUTINGS - 1:
                    # broadcast outputs to all 128 partitions via selT matmul
                    pob = ps_ef.tile([128, 160], F32, tag="pob", bufs=1)
                    nc.tensor.matmul(pob[:], lhsT=selT[:], rhs=outputs[:],
                                     start=True, stop=True)
                    ob = rp.tile([128, 160], F32, tag="ob", bufs=2)
                    nc.scalar.copy(ob[:], pob[:])
                    # tmp = u_hat * ob (ob broadcast over chunks)
                    for half in range(2):
                        lo = half * (NGRP // 2)
                        eng = nc.vector if half == 0 else nc.gpsimd
                        eng.tensor_tensor(
                            _sub(tmp[:], lo * 160,
                                 [[160, NGRP // 2], [1, 160]]),
                            _sub(uh[:], lo * 160,
                                 [[160, NGRP // 2], [1, 160]]),
                            _v(ob, [[0, NGRP // 2], [1, 160]]),
                            op=OP.mult)
                    # du = sum over dc (innermost 16) -> [128, NGRP*NUM_CAP]
                    du = rp.tile([128, NGRP * NUM_CAP], F32, tag="du", bufs=2)
                    nc.vector.tensor_reduce(
                        _v(du, [[NUM_CAP, NGRP], [1, NUM_CAP]]),
                        _v(tmp, [[160, NGRP], [DIM_CAP, NUM_CAP],
                                 [1, DIM_CAP]]),
                        axis=AX.X, op=OP.add)
                    nc.vector.tensor_add(bl_t[:], bl_t[:], du[:])

            if debug:
                nc.sync.dma_start(out=dbg_bl_d[:, :], in_=bl_t[:])
                nc.sync.dma_start(out=dbg_o_d[:, :], in_=outputs[:])

            # final linear
            pt1 = ps_ef.tile([128, BL], F32, tag="pt1", bufs=1)
            nc.tensor.matmul(pt1[:, :], lhsT=outputs[:, 0:128],
                             rhs=ident[:BL, :BL], start=True, stop=True)
            pt2 = ps_ef.tile([32, BL], F32, tag="pt2", bufs=1)
            nc.tensor.matmul(pt2[:, :], lhsT=outputs[:, 128:160],
                             rhs=ident[:BL, :BL], start=True, stop=True)
            capsT = rp.tile([128, 2 * BL], F32, tag="capsT")
            nc.vector.tensor_copy(capsT[:, 0:BL], pt1[:])
            nc.vector.tensor_copy(capsT[:32, BL:2 * BL], pt2[:])
            pf = ps_ef.tile([2, BL], F32, tag="pf", bufs=1)
            nc.tensor.matmul(pf[:], lhsT=wlin[:, 0, :], rhs=capsT[:, 0:BL],
                             start=True, stop=False)
            nc.tensor.matmul(pf[:], lhsT=wlin[:32, 1, :],
                             rhs=capsT[:32, BL:2 * BL],
                             start=False, stop=True)
            outT = rp.tile([2, BL], F32, tag="outT")
            nc.scalar.activation(outT[:], pf[:], AF.Identity,
                                 bias=blin[:, 0:1])
            dst = bass.AP(tensor=out_d, offset=0, ap=[[1, 2], [2, BL]])
            nc.sync.dma_start(out=dst, in_=outT[:])

    return nc


_CACHE = {}


def _get_nc(zero_bhn):
    if zero_bhn not in _CACHE:
        nc = _build(zero_bhn)
        _split_waits(nc)   # HW-path legalization (CoreSim path builds its own)
        _CACHE[zero_bhn] = nc
    return _CACHE[zero_bhn]


def _host_inputs(x, emb, w_ih_f, w_hh_f, b_ih_f, b_hh_f,
                 w_ih_b, w_hh_b, b_ih_b, b_hh_b, W_cap, W_lin, b_lin):
    """Build the per-core input maps (everything but xidx is shared)."""
    import ml_dtypes
    f32 = np.float32
    bf16 = np.float16
    neg = np.ones((G3,), f32)
    neg[H:2 * H] = -1.0        # negate z gate (sigmoid -> 1-z)

    wih = np.stack([(w_ih_f.T * neg).astype(bf16), (w_ih_b.T * neg).astype(bf16)])
    whh = np.stack([(w_hh_f.T * neg).astype(bf16), (w_hh_b.T * neg).astype(bf16)])

    biasx = np.zeros((128, 6), f32)
    for d, (bi, bh) in enumerate([(b_ih_f, b_hh_f), (b_ih_b, b_hh_b)]):
        biasx[:, _BLKRZ[(d, 0)]] = (bi[0:H] + bh[0:H])
        biasx[:, _BLKRZ[(d, 1)]] = -(bi[H:2 * H] + bh[H:2 * H])
        biasx[:, 4 + d] = bi[2 * H:3 * H]
    bhn = np.zeros((128, 2), f32)
    bhn[:, 0] = b_hh_f[2 * H:3 * H]
    bhn[:, 1] = b_hh_b[2 * H:3 * H]
    zero_bhn = bool(np.all(bhn == 0.0))

    wcap = np.stack([W_cap[0:H, :].astype(bf16), W_cap[H:2 * H, :].astype(bf16)])
    selB = (np.arange(128)[:, None] % BL == np.arange(BL)[None, :]).astype(f32)
    selT = selB.T.copy()
    ident = np.eye(128, dtype=f32)

    shared = dict(emb=np.ascontiguousarray(emb, f32), wih=wih, whh=whh,
                  biasx=biasx, bhn=bhn, wcap=wcap,
                  wlin=np.ascontiguousarray(W_lin, f32),
                  blin=np.ascontiguousarray(b_lin, f32).reshape(2, 1),
                  selB=selB, selT=selT, ident=ident)

    in_maps = []
    for c in range(NCORES):
        xl = np.asarray(x[c * BL:(c + 1) * BL, :])          # [BL, S]
        tok = xl.T.reshape(-1).astype(np.int32)             # s-major [NTOK]
        xidx = np.ascontiguousarray(tok.reshape(NGRP, 128).T)  # [128, NGRP]
        in_maps.append(dict(shared, xidx=xidx))
    return in_maps, zero_bhn


def kernel(**inputs):
    in_maps, zero_bhn = _host_inputs(**{k: np.asarray(v) for k, v in
                                        inputs.items()})
    nc = _get_nc(zero_bhn)
    res = run_bass_kernel_spmd(nc, in_maps, list(range(NCORES)))
    return np.concatenate([res.results[c]["out"] for c in range(NCORES)],
                          axis=0)


def _install_ntff_hook():
    """Shim the missing antenv.axon_hooks so trace=True works under axon."""
    import sys, types
    if "antenv.axon_hooks" in sys.modules:
        return
    mod = types.ModuleType("antenv.axon_hooks")
    _h = [None]
    mod.set_axon_ntff_profile_hook = lambda h: _h.__setitem__(0, h)
    mod.get_axon_ntff_profile_hook = lambda: _h[0]
    sys.modules["antenv.axon_hooks"] = mod
    import antenv
    antenv.axon_hooks = mod
    from trn_agent_boot.trn_boot import _ntff_profile_via_ctypes
    mod.set_axon_ntff_profile_hook(
        _ntff_profile_via_ctypes("/opt/axon/libaxon_pjrt.so"))


def kernel_profiled(**inputs):
    """Same as kernel() but with NTFF tracing; returns (out, result_obj)."""
    _install_ntff_hook()
    in_maps, zero_bhn = _host_inputs(**{k: np.asarray(v) for k, v in
                                        inputs.items()})
    nc = _get_nc(zero_bhn)
    res = run_bass_kernel_spmd(nc, in_maps, list(range(NCORES)), trace=True)
    out = np.concatenate([res.results[c]["out"] for c in range(NCORES)],
                         axis=0)
    return out, res
